# revision 23
# baseline (speedup 1.0000x reference)
"""GAT (2-layer graph attention network) on 8 Trainium2 NeuronCores.

Strategy: node partition. Core c owns nodes [c*6250, (c+1)*6250) and all edges
whose src lies in its range (segment sums in the reference are over src).

Layer-1 node table (h|er) is computed REPLICATED on every core (the dense
matmul for all 50000 nodes is ~16us of PE time), which removes the first
AllGather entirely. The layer-2 table still needs an AllGather (it depends on
the sparse phase output).

Sparse phase: per 128-src-node tile, edges packed into 128-row chunks. h[dst]
rows are fetched with dma_gather (2 per tile: table split in two 25000-row
halves so indices fit int16), attention coefficients via mask matmuls
(one-hot src masks), segment sums as mask-matmuls accumulated in PSUM.

Self-contained: only needs numpy + the concourse (Bass) stack at
/opt/trn_rl_repo. All shapes hardcoded for the nn_GAT problem.
"""
import sys

if "/opt/trn_rl_repo" not in sys.path:
    sys.path.insert(0, "/opt/trn_rl_repo")

import numpy as np

import concourse.bacc as bacc
import concourse.bass as bass
import concourse.mybir as mybir
import concourse.tile as tile
from concourse.bass_utils import run_bass_kernel_spmd
from concourse.masks import make_identity

# problem shapes
N = 50000
E = 800000
FIN = 256
H = 8          # heads, layer 1
F1 = 32        # per-head features, layer 1
NH = 256       # hidden = H*F1
C = 47         # classes
NCORES = 8
NPC = N // NCORES          # nodes per core = 6250
T = (NPC + 127) // 128     # src tiles per core = 49
LAST_ROWS = NPC - (T - 1) * 128   # rows in last tile = 106
NPAD = T * 128             # padded node count per core = 6272
TG = (N + 127) // 128      # global node tiles = 391
LASTG = N - (TG - 1) * 128  # rows in last global tile = 80
NGPAD = TG * 128           # padded global node count = 50048
HALF = 25000               # table split point (int16 gather indices)
ROW = 64                   # table row f32 elements (256B, dma_gather granule)

F32 = mybir.dt.float32
F16 = mybir.dt.float16
F8 = mybir.dt.float8e4
I16 = mybir.dt.int16

ALU = mybir.AluOpType
ACT = mybir.ActivationFunctionType

_cache = {}


def _preprocess(edge_src, edge_dst):
    """Sort/group/pad edges; build per-core index/mask arrays (numpy only).

    Per core, per src tile: edges split into A (dst < HALF) and B (dst >=
    HALF) sections, each padded to a multiple of 128 (chunk counts cA/cB are
    maxed across cores so all cores run the same program). Edge slot (p, k)
    <-> flat position k*128+p.
    """
    order = np.argsort(edge_src, kind="stable")
    src_s = np.asarray(edge_src)[order].astype(np.int64)
    dst_s = np.asarray(edge_dst)[order].astype(np.int64)
    bounds = np.searchsorted(src_s, np.arange(NCORES + 1) * NPC)

    # gather per-(core, tile) A/B edge lists
    AB = []  # [core][tile] -> (srcA, dstA, srcB, dstB)  (src local to tile)
    for c in range(NCORES):
        b0, b1 = bounds[c], bounds[c + 1]
        sc = src_s[b0:b1] - c * NPC
        dc = dst_s[b0:b1]
        tloc = sc >> 7
        rows = []
        for t in range(T):
            m = tloc == t
            st, dt_ = sc[m] - t * 128, dc[m]
            a = dt_ < HALF
            rows.append((st[a], dt_[a], st[~a], dt_[~a] - HALF))
        AB.append(rows)

    cA = [max(1, max((len(AB[c][t][0]) + 127) // 128 for c in range(NCORES)))
          for t in range(T)]
    cB = [max(1, max((len(AB[c][t][2]) + 127) // 128 for c in range(NCORES)))
          for t in range(T)]
    cpt = [cA[t] + cB[t] for t in range(T)]
    scols = sum(cpt)                    # total chunks
    sidx_cols = scols * 8               # int16 strip columns (128 idx -> 8)

    import ml_dtypes
    cpt_off = np.concatenate([[0], np.cumsum(cpt)]).astype(int)
    ins = []
    jj = np.arange(128)
    for c in range(NCORES):
        sidx = np.zeros((16, sidx_cols), dtype=np.int16)
        srcloc = np.full((128, scols), 1000, dtype=np.int32)
        srcflat = np.full((scols * 128,), 1000, dtype=np.int32)
        off = 0
        for t in range(T):
            sA, dA, sB, dB = AB[c][t]
            for (ss, dd, nk) in ((sA, dA, cA[t]), (sB, dB, cB[t])):
                ni = nk * 128
                dpad = np.zeros(ni, dtype=np.int64)
                dpad[:len(dd)] = dd
                # index strip: position i at (i%16, i//16)
                sidx[:, off * 8:(off + nk) * 8] = \
                    dpad.astype(np.int16).reshape(-1, 16).T
                i = np.arange(len(ss))
                srcloc[i % 128, off + i // 128] = ss
                srcflat[off * 128 + i] = ss
                off += nk
        # fp8 one-hot masks (1.0 in e4m3 is 0x38):
        #   mskT[p, kj] = (src of edge slot kj == p)
        #   msk[p, k, j] = (srcloc[p, k] == j)
        mskT = (srcflat[None, :] == jj[:, None]).astype(np.uint8) * 0x38
        msk = (srcloc[:, :, None] == jj[None, None, :]).astype(np.uint8) * 0x38
        # combined per-tile layout: [mskT(ck*128) | msk(ck*128)]
        masks = np.zeros((128, scols * 256), dtype=np.uint8)
        for t in range(T):
            o, ck = cpt_off[t], cpt[t]
            base = o * 256
            masks[:, base:base + ck * 128] = mskT[:, o * 128:(o + ck) * 128]
            masks[:, base + ck * 128:base + ck * 256] = \
                msk[:, o:o + ck, :].reshape(128, ck * 128)
        ins.append({
            "sidx": np.tile(sidx, (8, 1)),
            "masks": masks.view(ml_dtypes.float8_e4m3),
        })
    return ins, tuple(cA), tuple(cB)


def _build(cA, cB, ablate_gather=False, ablate_coll=False):
    T_ = len(cA)
    cpt = [cA[t] + cB[t] for t in range(T_)]
    cptmax = max(cpt)
    scols = sum(cpt)
    sidx_cols = scols * 8
    # per-tile offsets into the concatenated strips
    off = np.concatenate([[0], np.cumsum(cpt)]).astype(int)

    nc = bacc.Bacc("TRN2", target_bir_lowering=False, debug=False,
                   num_devices=NCORES, num_swdge_queues=4)

    # ---- external inputs (per core) ----
    d_xTf = nc.dram_tensor("xTf", [2, 128, NGPAD], F16, kind="ExternalInput")
    d_xTo = nc.dram_tensor("xTo", [2, 128, NPAD], F16, kind="ExternalInput")
    d_W1 = nc.dram_tensor("W1cat", [128, 2, 48], F16, kind="ExternalInput")
    d_W2 = nc.dram_tensor("W2cat", [128, 2, 49], F16, kind="ExternalInput")
    d_b1 = nc.dram_tensor("b1f", [128, NH], F32, kind="ExternalInput")
    d_b2 = nc.dram_tensor("b2f", [128, C], F32, kind="ExternalInput")
    d_sidx = nc.dram_tensor("sidx", [128, sidx_cols], I16, kind="ExternalInput")
    d_masks = nc.dram_tensor("masks", [128, scols * 256], F8,
                             kind="ExternalInput")

    d_out = nc.dram_tensor("out", [NPC, C], F32, kind="ExternalOutput")

    # ---- internal DRAM tables ----
    d_t1 = nc.dram_tensor("t1", [N, ROW], F32)              # [h | er | pad]
    d_t2loc = nc.dram_tensor("t2loc", [NPC, ROW], F32)      # [h2 | er2 | pad]
    d_t2 = nc.dram_tensor("t2", [N, ROW], F32, addr_space="Shared")

    groups = [list(range(NCORES))]

    # SWDGE queue must track Tile's DMASW sem-lane round-robin (lane =
    # pool-DMA index % 8, queue = index % 4) or the sem gets updated from a
    # queue it isn't locked to.
    swq = iter(range(1 << 30))

    def gather(out_ap, table_ap, t, half):
        nk = (cA[t] if half == 0 else cB[t])
        o8 = (off[t] + (0 if half == 0 else cA[t])) * 8
        ni = nk * 128
        # single_packet=True wedges the SWDGE ucode above ~1024 idxs
        return nc.gpsimd.dma_gather(
            out_ap=out_ap, in_ap=table_ap,
            idxs_ap=sidx_sb[:, o8:o8 + nk * 8],
            num_idxs=ni, num_idxs_reg=ni, elem_size=ROW,
            single_packet=False, queue_num=next(swq) % 4)

    with tile.TileContext(nc, num_cores=NCORES) as tc:
        with (
            tc.tile_pool(name="const", bufs=1) as cpool,
            tc.tile_pool(name="rt", bufs=1) as rtpool,
            tc.tile_pool(name="work", bufs=2) as wp,
            tc.tile_pool(name="small", bufs=3) as sp,
            tc.tile_pool(name="psA", bufs=2, space="PSUM") as psA,   # agg
            tc.tile_pool(name="psD", bufs=2, space="PSUM") as psD,   # dense
            tc.tile_pool(name="psT", bufs=2, space="PSUM") as psT,   # transposes
            tc.tile_pool(name="psE", bufs=2, space="PSUM") as psE,   # el expand
        ):
            # ---------- constants ----------
            b1sb = cpool.tile([128, NH], F32)
            nc.sync.dma_start(out=b1sb[:], in_=d_b1.ap())
            b2sb = cpool.tile([128, C], F32)
            nc.sync.dma_start(out=b2sb[:], in_=d_b2.ap())
            ident = cpool.tile([128, 128], F16)
            make_identity(nc, ident[:])
            W1cat = cpool.tile([128, 2, 48], F16)
            nc.sync.dma_start(out=W1cat[:], in_=d_W1.ap())
            W2cat = cpool.tile([128, 2, 49], F16)
            nc.sync.dma_start(out=W2cat[:], in_=d_W2.ap())
            sidx_sb = cpool.tile([128, sidx_cols], I16)
            nc.sync.dma_start(out=sidx_sb[:], in_=d_sidx.ap())

            rT = rtpool.tile([128, 2, NPAD], F16)   # transposed post-elu l1 out
            el2sb = rtpool.tile([128, T_], F32)     # own layer-2 el, per tile

            # ---------- phase D1 (replicated): h/er table for ALL nodes ----
            G4 = 4  # node tiles per group
            ngroups = (TG + G4 - 1) // G4
            for g in range(ngroups):
                tiles = min(G4, TG - g * G4)
                ncols = tiles * 128
                xa = wp.tile([128, 2, G4 * 128], F16, tag="xa")
                nc.sync.dma_start(
                    out=xa[:, :, 0:ncols],
                    in_=d_xTf.ap()[:, :, g * G4 * 128:g * G4 * 128 + ncols]
                    .rearrange("q p n -> p q n"))
                h4 = wp.tile([128, G4, ROW], F32, tag="h4")
                for j in range(tiles):
                    ps = psD.tile([128, 48], F32, tag="dense")
                    for q in range(2):
                        nc.tensor.matmul(out=ps[:],
                                         lhsT=xa[:, q, j * 128:(j + 1) * 128],
                                         rhs=W1cat[:, q, :],
                                         start=q == 0, stop=q == 1)
                    nc.scalar.copy(out=h4[:, j, 0:48], in_=ps[:])
                base = g * G4 * 128
                rows = min(N - base, ncols)
                full_j = rows // 128
                if full_j:
                    nc.sync.dma_start(
                        out=d_t1.ap()[base:base + full_j * 128, 0:48]
                        .rearrange("(j p) f -> p j f", p=128),
                        in_=h4[:, 0:full_j, 0:48])
                rem = rows - full_j * 128
                if rem:
                    nc.sync.dma_start(
                        out=d_t1.ap()[base + full_j * 128:base + rows, 0:48],
                        in_=h4[0:rem, full_j, 0:48])
            tc.strict_bb_all_engine_barrier()

            # ---------- phase S1 (+ fused D2) ----------
            for t in range(T_):
                rows = 128 if t < T_ - 1 else LAST_ROWS
                ck = cpt[t]

                G1 = wp.tile([128, cptmax, ROW], F32, tag="G1")
                if ablate_gather:
                    nc.sync.dma_start(
                        out=G1[:, 0:ck, :], in_=d_t1.ap()[0:128, :]
                        .rearrange("p f -> p () f").to_broadcast([128, ck, ROW]))
                else:
                    gather(G1[:, 0:cA[t], :], d_t1.ap(), t, 0)
                    gather(G1[:, cA[t]:ck, :], d_t1.ap()[HALF:N, :], t, 1)

                # own el for this tile: el = x @ (W1 @ Wl1)  (cols 40:48)
                xo = sp.tile([128, 2, 128], F16, tag="xo")
                nc.sync.dma_start(
                    out=xo[:],
                    in_=d_xTo.ap()[:, :, t * 128:(t + 1) * 128]
                    .rearrange("q p n -> p q n"))
                pel = psD.tile([128, 8], F32, tag="dense")
                for q in range(2):
                    nc.tensor.matmul(out=pel[:], lhsT=xo[:, q, :],
                                     rhs=W1cat[:, q, 40:48],
                                     start=q == 0, stop=q == 1)
                elt = sp.tile([128, H], F16, tag="elt")
                nc.vector.tensor_copy(out=elt[:], in_=pel[:])

                # fp8 masks: [mskT(ck*128) | msk(ck*128)] in one load
                mts = wp.tile([128, cptmax * 256], F8, tag="mts")
                nc.sync.dma_start(
                    out=mts[:, 0:ck * 256],
                    in_=d_masks.ap()[:, off[t] * 256:off[t] * 256 + ck * 256])
                pse = psE.tile([128, cptmax * H], F32, tag="elexp")
                for k in range(ck):
                    nc.tensor.matmul(out=pse[:, k * H:(k + 1) * H],
                                     lhsT=mts[:, k * 128:(k + 1) * 128],
                                     rhs=elt[:],
                                     start=k == 0, stop=k == ck - 1,
                                     skip_group_check=True)

                # s = er[dst] + el[src]; leaky; exp
                s = wp.tile([128, cptmax, H], F32, tag="s")
                nc.vector.tensor_tensor(
                    out=s[:, 0:ck, :], in0=G1[:, 0:ck, 32:40],
                    in1=pse[:, 0:ck * H].rearrange("p (k h) -> p k h", h=H),
                    op=ALU.add)
                sL = wp.tile([128, cptmax, H], F32, tag="sL")
                nc.vector.scalar_tensor_tensor(
                    out=sL[:, 0:ck, :], in0=s[:, 0:ck, :], scalar=0.2,
                    in1=s[:, 0:ck, :], op0=ALU.mult, op1=ALU.max)
                rhs = wp.tile([128, cptmax, 264], F16, tag="rhs")
                nc.scalar.activation(out=rhs[:, 0:ck, 256:264],
                                     in_=sL[:, 0:ck, :], func=ACT.Exp)

                hdb = wp.tile([128, cptmax, F1], F16, tag="hdb")
                nc.scalar.copy(out=hdb[:, 0:ck, :], in_=G1[:, 0:ck, 0:32])

                nc.gpsimd.tensor_tensor(
                    out=rhs[:, 0:ck, 0:256].rearrange(
                        "p k (h f) -> p k h f", h=H),
                    in0=rhs[:, 0:ck, 256:264].rearrange(
                        "p k (h o) -> p k h o", o=1)
                    .to_broadcast([128, ck, H, F1]),
                    in1=hdb[:, 0:ck, :].rearrange(
                        "p k (o f) -> p k o f", o=1)
                    .to_broadcast([128, ck, H, F1]),
                    op=ALU.mult)

                ps1 = psA.tile([128, 264], F32, tag="agg")
                for k in range(ck):
                    nc.tensor.matmul(out=ps1[:],
                                     lhsT=mts[:, (ck + k) * 128:(ck + k + 1) * 128],
                                     rhs=rhs[:, k, :],
                                     start=k == 0, stop=k == ck - 1)

                # epilogue: out1 = agg/denom + b1 ; r = elu(out1); rT = r.T
                dn = sp.tile([128, H], F32, tag="dn")
                nc.vector.tensor_scalar(out=dn[:], in0=ps1[:, 256:264],
                                        scalar1=1e-12, scalar2=None, op0=ALU.max)
                rc = sp.tile([128, H], F32, tag="rc")
                nc.vector.reciprocal(out=rc[:], in_=dn[:])
                o1 = wp.tile([128, NH], F32, tag="o1")
                nc.vector.tensor_tensor(
                    out=o1[:].rearrange("p (h f) -> p h f", h=H),
                    in0=ps1[:, 0:256].rearrange("p (h f) -> p h f", h=H),
                    in1=rc[:].rearrange("p (h o) -> p h o", o=1)
                    .to_broadcast([128, H, F1]),
                    op=ALU.mult)
                o1b = wp.tile([128, NH], F32, tag="o1b")
                nc.vector.tensor_tensor(out=o1b[:], in0=o1[:], in1=b1sb[:],
                                        op=ALU.add)
                # elu(x) = max(x,0)-1 + min(exp(x),1)
                ex = wp.tile([128, NH], F32, tag="ex")
                nc.scalar.activation(out=ex[:], in_=o1b[:], func=ACT.Exp)
                p1 = wp.tile([128, NH], F32, tag="p1")
                nc.vector.tensor_scalar(out=p1[:], in0=o1b[:], scalar1=0.0,
                                        scalar2=-1.0, op0=ALU.max, op1=ALU.add)
                r_ = wp.tile([128, NH], F16, tag="r_")
                nc.vector.scalar_tensor_tensor(out=r_[:], in0=ex[:], scalar=1.0,
                                               in1=p1[:], op0=ALU.min,
                                               op1=ALU.add)
                for q in range(2):
                    pt = psT.tile([128, 128], F16, tag="pt")
                    nc.tensor.transpose(out=pt[:], in_=r_[:, q * 128:(q + 1) * 128],
                                        identity=ident[:])
                    if q == 0:
                        nc.scalar.copy(out=rT[:, q, t * 128:(t + 1) * 128],
                                       in_=pt[:])
                    else:
                        nc.vector.tensor_copy(out=rT[:, q, t * 128:(t + 1) * 128],
                                              in_=pt[:])

                # D2: h2/er2/el2 for this tile
                ps2 = psD.tile([128, 49], F32, tag="dense")
                for q in range(2):
                    nc.tensor.matmul(out=ps2[:], lhsT=rT[:, q, t * 128:(t + 1) * 128],
                                     rhs=W2cat[:, q, :], start=q == 0, stop=q == 1)
                h2sb = wp.tile([128, ROW], F32, tag="h2sb")
                nc.scalar.copy(out=h2sb[:, 0:48], in_=ps2[:, 0:48])
                nc.scalar.copy(out=el2sb[:, t:t + 1], in_=ps2[:, 48:49])
                nc.sync.dma_start(out=d_t2loc.ap()[t * 128:t * 128 + rows, 0:48],
                                  in_=h2sb[0:rows, 0:48])

            # ---------- C2: share layer-2 table ----------
            if ablate_coll:
                nc.sync.dma_start(out=d_t2.ap()[0:NPC, :], in_=d_t2loc.ap())
            else:
                nc.gpsimd.collective_compute(
                    "AllGather", ALU.bypass, replica_groups=groups,
                    ins=[d_t2loc.ap()], outs=[d_t2.ap()])
            tc.strict_bb_all_engine_barrier()

            # ---------- phase S2 ----------
            for t in range(T_):
                rows = 128 if t < T_ - 1 else LAST_ROWS
                ck = cpt[t]

                Gt = wp.tile([128, cptmax, ROW], F32, tag="Gt")
                if ablate_gather:
                    nc.sync.dma_start(
                        out=Gt[:, 0:ck, :], in_=d_t2.ap()[0:128, :]
                        .rearrange("p f -> p () f").to_broadcast([128, ck, ROW]))
                else:
                    gather(Gt[:, 0:cA[t], :], d_t2.ap(), t, 0)
                    gather(Gt[:, cA[t]:ck, :], d_t2.ap()[HALF:N, :], t, 1)

                mts = wp.tile([128, cptmax * 256], F8, tag="mts")
                nc.sync.dma_start(
                    out=mts[:, 0:ck * 256],
                    in_=d_masks.ap()[:, off[t] * 256:off[t] * 256 + ck * 256])
                el2t = sp.tile([128, 1], F16, tag="el2t")
                nc.vector.tensor_copy(out=el2t[:], in_=el2sb[:, t:t + 1])
                pse2 = psE.tile([128, cptmax], F32, tag="elexp")
                for k in range(ck):
                    nc.tensor.matmul(out=pse2[:, k:k + 1],
                                     lhsT=mts[:, k * 128:(k + 1) * 128],
                                     rhs=el2t[:],
                                     start=k == 0, stop=k == ck - 1,
                                     skip_group_check=True)

                s2 = sp.tile([128, cptmax], F32, tag="s2")
                nc.vector.tensor_tensor(
                    out=s2[:, 0:ck],
                    in0=Gt[:, 0:ck, 47:48].rearrange("p k o -> p (k o)"),
                    in1=pse2[:, 0:ck], op=ALU.add)
                sL2 = sp.tile([128, cptmax], F32, tag="sL2")
                nc.vector.scalar_tensor_tensor(
                    out=sL2[:, 0:ck], in0=s2[:, 0:ck], scalar=0.2,
                    in1=s2[:, 0:ck], op0=ALU.mult, op1=ALU.max)
                rhs2 = wp.tile([128, cptmax, 48], F16, tag="rhs2")
                nc.scalar.activation(
                    out=rhs2[:, 0:ck, 47:48].rearrange("p k o -> p (k o)"),
                    in_=sL2[:, 0:ck], func=ACT.Exp)

                nc.vector.tensor_tensor(
                    out=rhs2[:, 0:ck, 0:47], in0=Gt[:, 0:ck, 0:47],
                    in1=rhs2[:, 0:ck, 47:48].to_broadcast([128, ck, C]),
                    op=ALU.mult)
                ps3 = psA.tile([128, 48], F32, tag="agg")
                for k in range(ck):
                    nc.tensor.matmul(out=ps3[:],
                                     lhsT=mts[:, (ck + k) * 128:(ck + k + 1) * 128],
                                     rhs=rhs2[:, k, :],
                                     start=k == 0, stop=k == ck - 1)

                # epilogue: out2 = agg2/denom2 + b2, then log_softmax
                dn2 = sp.tile([128, 1], F32, tag="dn2")
                nc.vector.tensor_scalar(out=dn2[:], in0=ps3[:, 47:48],
                                        scalar1=1e-12, scalar2=None, op0=ALU.max)
                rc2 = sp.tile([128, 1], F32, tag="rc2")
                nc.vector.reciprocal(out=rc2[:], in_=dn2[:])
                o2b = wp.tile([128, C], F32, tag="o2b")
                nc.vector.scalar_tensor_tensor(out=o2b[:], in0=ps3[:, 0:47],
                                               scalar=rc2[:, 0:1], in1=b2sb[:],
                                               op0=ALU.mult, op1=ALU.add)
                mx = sp.tile([128, 1], F32, tag="mx")
                nc.vector.tensor_reduce(out=mx[:], in_=o2b[:],
                                        axis=mybir.AxisListType.X, op=ALU.max)
                xm = wp.tile([128, C], F32, tag="xm")
                nc.vector.tensor_scalar(out=xm[:], in0=o2b[:], scalar1=mx[:, 0:1],
                                        scalar2=None, op0=ALU.subtract)
                ex2 = wp.tile([128, C], F32, tag="ex2")
                se = sp.tile([128, 1], F32, tag="se")
                nc.scalar.activation(out=ex2[:], in_=xm[:], func=ACT.Exp,
                                     accum_out=se[:])
                ls = sp.tile([128, 1], F32, tag="ls")
                nc.scalar.activation(out=ls[:], in_=se[:], func=ACT.Ln)
                fin = wp.tile([128, C], F32, tag="fin")
                nc.vector.tensor_scalar(out=fin[:], in0=xm[:], scalar1=ls[:, 0:1],
                                        scalar2=None, op0=ALU.subtract)
                nc.sync.dma_start(out=d_out.ap()[t * 128:t * 128 + rows, :],
                                  in_=fin[0:rows, :])

    nc.compile()
    return nc


def _make_inputs(x, edge_src, edge_dst, W1, Wl1, Wr1, b1, W2, Wl2, Wr2, b2):
    edge_ins, cA, cB = _preprocess(edge_src, edge_dst)
    x = np.asarray(x, dtype=np.float32)
    W1 = np.asarray(W1, dtype=np.float32)
    Wl1 = np.asarray(Wl1, dtype=np.float32)
    Wr1 = np.asarray(Wr1, dtype=np.float32)
    W2 = np.asarray(W2, dtype=np.float32)
    Wl2 = np.asarray(Wl2, dtype=np.float32)
    Wr2 = np.asarray(Wr2, dtype=np.float32)

    W1cat = np.zeros((128, 2, 48), dtype=np.float16)
    W1cat[:, :, 0:32] = W1.reshape(2, 128, F1).transpose(1, 0, 2)
    W1cat[:, :, 32:40] = (W1 @ Wr1).reshape(2, 128, H).transpose(1, 0, 2)
    W1cat[:, :, 40:48] = (W1 @ Wl1).reshape(2, 128, H).transpose(1, 0, 2)
    W2cat = np.zeros((128, 2, 49), dtype=np.float16)
    W2cat[:, :, 0:47] = W2.reshape(2, 128, C).transpose(1, 0, 2)
    W2cat[:, :, 47:48] = (W2 @ Wr2).reshape(2, 128, 1).transpose(1, 0, 2)
    W2cat[:, :, 48:49] = (W2 @ Wl2).reshape(2, 128, 1).transpose(1, 0, 2)

    b1f = np.tile(np.tile(np.asarray(b1, np.float32), H)[None, :], (128, 1))
    b2f = np.tile(np.asarray(b2, np.float32)[None, :], (128, 1))

    xTf = np.zeros((2, 128, NGPAD), dtype=np.float16)
    xTf[:, :, :N] = np.ascontiguousarray(x.T).reshape(2, 128, N)

    common = {
        "xTf": xTf, "W1cat": W1cat, "W2cat": W2cat,
        "b1f": b1f, "b2f": b2f,
    }
    in_maps = []
    for c in range(NCORES):
        xTo = np.zeros((2, 128, NPAD), dtype=np.float16)
        xs = np.ascontiguousarray(x[c * NPC:(c + 1) * NPC].T)   # [256, NPC]
        xTo[:, :, :NPC] = xs.reshape(2, 128, NPC)
        m = dict(common)
        m["xTo"] = xTo
        m.update(edge_ins[c])
        in_maps.append(m)
    return in_maps, (cA, cB)


def _run(inputs, trace=False, **build_kw):
    in_maps, key = _make_inputs(**inputs)
    ck = (key, tuple(sorted(build_kw.items())))
    if ck not in _cache:
        _cache[ck] = _build(*key, **build_kw)
    nc = _cache[ck]
    bkr = run_bass_kernel_spmd(nc, in_maps, list(range(NCORES)), trace=trace)
    out = np.concatenate([bkr.results[c]["out"] for c in range(NCORES)], axis=0)
    return out.astype(np.float32), bkr


def kernel(**inputs):
    out, _ = _run(inputs, trace=False)
    return out


# revision 24
# speedup vs baseline: 1.2639x; 1.2639x over previous
"""GAT (2-layer graph attention network) on 8 Trainium2 NeuronCores.

Strategy: node partition. Core c owns nodes [c*6250, (c+1)*6250) and all edges
whose src lies in its range (segment sums in the reference are over src).

Layer-1 node table (h|er) is computed REPLICATED on every core (the dense
matmul for all 50000 nodes is ~16us of PE time), which removes the first
AllGather entirely. The layer-2 table still needs an AllGather (it depends on
the sparse phase output).

Sparse phase: per 128-src-node tile, edges packed into 128-row chunks. h[dst]
rows are fetched with dma_gather (2 per tile: table split in two 25000-row
halves so indices fit int16), attention coefficients via mask matmuls
(one-hot src masks), segment sums as mask-matmuls accumulated in PSUM.

Self-contained: only needs numpy + the concourse (Bass) stack at
/opt/trn_rl_repo. All shapes hardcoded for the nn_GAT problem.
"""
import sys

if "/opt/trn_rl_repo" not in sys.path:
    sys.path.insert(0, "/opt/trn_rl_repo")

import numpy as np

import concourse.bacc as bacc
import concourse.bass as bass
import concourse.mybir as mybir
import concourse.tile as tile
from concourse.bass_utils import run_bass_kernel_spmd
from concourse.masks import make_identity

# problem shapes
N = 50000
E = 800000
FIN = 256
H = 8          # heads, layer 1
F1 = 32        # per-head features, layer 1
NH = 256       # hidden = H*F1
C = 47         # classes
NCORES = 8
NPC = N // NCORES          # nodes per core = 6250
T = (NPC + 127) // 128     # src tiles per core = 49
LAST_ROWS = NPC - (T - 1) * 128   # rows in last tile = 106
NPAD = T * 128             # padded node count per core = 6272
TG = (N + 127) // 128      # global node tiles = 391
LASTG = N - (TG - 1) * 128  # rows in last global tile = 80
NGPAD = TG * 128           # padded global node count = 50048
HALF = 25000               # table split point (int16 gather indices)
ROW = 64                   # table row f32 elements (256B, dma_gather granule)

F32 = mybir.dt.float32
F16 = mybir.dt.float16
F8 = mybir.dt.float8e4
I16 = mybir.dt.int16

ALU = mybir.AluOpType
ACT = mybir.ActivationFunctionType

_cache = {}


def _preprocess(edge_src, edge_dst):
    """Sort/group/pad edges; build per-core index/mask arrays (numpy only).

    Per core, per src tile: edges split into A (dst < HALF) and B (dst >=
    HALF) sections, each padded to a multiple of 128 (chunk counts cA/cB are
    maxed across cores so all cores run the same program). Edge slot (p, k)
    <-> flat position k*128+p.
    """
    order = np.argsort(edge_src, kind="stable")
    src_s = np.asarray(edge_src)[order].astype(np.int64)
    dst_s = np.asarray(edge_dst)[order].astype(np.int64)
    bounds = np.searchsorted(src_s, np.arange(NCORES + 1) * NPC)

    # gather per-(core, tile) A/B edge lists
    AB = []  # [core][tile] -> (srcA, dstA, srcB, dstB)  (src local to tile)
    for c in range(NCORES):
        b0, b1 = bounds[c], bounds[c + 1]
        sc = src_s[b0:b1] - c * NPC
        dc = dst_s[b0:b1]
        tloc = sc >> 7
        rows = []
        for t in range(T):
            m = tloc == t
            st, dt_ = sc[m] - t * 128, dc[m]
            a = dt_ < HALF
            rows.append((st[a], dt_[a], st[~a], dt_[~a] - HALF))
        AB.append(rows)

    cA = [max(1, max((len(AB[c][t][0]) + 127) // 128 for c in range(NCORES)))
          for t in range(T)]
    cB = [max(1, max((len(AB[c][t][2]) + 127) // 128 for c in range(NCORES)))
          for t in range(T)]
    cpt = [cA[t] + cB[t] for t in range(T)]
    scols = sum(cpt)                    # total chunks
    sidx_cols = scols * 8               # int16 strip columns (128 idx -> 8)

    import ml_dtypes
    cpt_off = np.concatenate([[0], np.cumsum(cpt)]).astype(int)
    ins = []
    jj = np.arange(128)
    for c in range(NCORES):
        sidx = np.zeros((16, sidx_cols), dtype=np.int16)
        srcloc = np.full((128, scols), 1000, dtype=np.int32)
        srcflat = np.full((scols * 128,), 1000, dtype=np.int32)
        off = 0
        for t in range(T):
            sA, dA, sB, dB = AB[c][t]
            for (ss, dd, nk) in ((sA, dA, cA[t]), (sB, dB, cB[t])):
                ni = nk * 128
                dpad = np.zeros(ni, dtype=np.int64)
                dpad[:len(dd)] = dd
                # index strip: position i at (i%16, i//16)
                sidx[:, off * 8:(off + nk) * 8] = \
                    dpad.astype(np.int16).reshape(-1, 16).T
                i = np.arange(len(ss))
                srcloc[i % 128, off + i // 128] = ss
                srcflat[off * 128 + i] = ss
                off += nk
        # fp8 one-hot masks (1.0 in e4m3 is 0x38):
        #   mskT[p, kj] = (src of edge slot kj == p)
        #   msk[p, k, j] = (srcloc[p, k] == j)
        mskT = (srcflat[None, :] == jj[:, None]).astype(np.uint8) * 0x38
        msk = (srcloc[:, :, None] == jj[None, None, :]).astype(np.uint8) * 0x38
        # combined per-tile layout: [mskT(ck*128) | msk(ck*128)]
        masks = np.zeros((128, scols * 256), dtype=np.uint8)
        for t in range(T):
            o, ck = cpt_off[t], cpt[t]
            base = o * 256
            masks[:, base:base + ck * 128] = mskT[:, o * 128:(o + ck) * 128]
            masks[:, base + ck * 128:base + ck * 256] = \
                msk[:, o:o + ck, :].reshape(128, ck * 128)
        ins.append({
            "sidx": np.tile(sidx, (8, 1)),
            "masks": masks.view(ml_dtypes.float8_e4m3),
        })
    return ins, tuple(cA), tuple(cB)


def _build(cA, cB, ablate_gather=False, ablate_coll=False):
    T_ = len(cA)
    cpt = [cA[t] + cB[t] for t in range(T_)]
    cptmax = max(cpt)
    scols = sum(cpt)
    sidx_cols = scols * 8
    # per-tile offsets into the concatenated strips
    off = np.concatenate([[0], np.cumsum(cpt)]).astype(int)

    nc = bacc.Bacc("TRN2", target_bir_lowering=False, debug=False,
                   num_devices=NCORES, num_swdge_queues=4)

    # ---- external inputs (per core) ----
    d_xTf = nc.dram_tensor("xTf", [2, 128, NGPAD], F16, kind="ExternalInput")
    d_xTo = nc.dram_tensor("xTo", [2, 128, NPAD], F16, kind="ExternalInput")
    d_W1 = nc.dram_tensor("W1cat", [128, 2, 48], F16, kind="ExternalInput")
    d_W2 = nc.dram_tensor("W2cat", [128, 2, 49], F16, kind="ExternalInput")
    d_b1 = nc.dram_tensor("b1f", [128, NH], F32, kind="ExternalInput")
    d_b2 = nc.dram_tensor("b2f", [128, C], F32, kind="ExternalInput")
    d_sidx = nc.dram_tensor("sidx", [128, sidx_cols], I16, kind="ExternalInput")
    d_masks = nc.dram_tensor("masks", [128, scols * 256], F8,
                             kind="ExternalInput")

    d_out = nc.dram_tensor("out", [NPC, C], F32, kind="ExternalOutput")

    # ---- internal DRAM tables ----
    d_t1 = nc.dram_tensor("t1", [N, ROW], F32)              # [h | er | pad]
    d_t2loc = nc.dram_tensor("t2loc", [NPC, ROW], F32)      # [h2 | er2 | pad]
    d_t2 = nc.dram_tensor("t2", [N, ROW], F32, addr_space="Shared")

    groups = [list(range(NCORES))]

    # SWDGE queue must track Tile's DMASW sem-lane round-robin (lane =
    # pool-DMA index % 8, queue = index % 4) or the sem gets updated from a
    # queue it isn't locked to.
    swq = iter(range(1 << 30))

    def gather(out_ap, table_ap, t, half):
        nk = (cA[t] if half == 0 else cB[t])
        o8 = (off[t] + (0 if half == 0 else cA[t])) * 8
        ni = nk * 128
        # single_packet=True wedges the SWDGE ucode above ~1024 idxs
        return nc.gpsimd.dma_gather(
            out_ap=out_ap, in_ap=table_ap,
            idxs_ap=sidx_sb[:, o8:o8 + nk * 8],
            num_idxs=ni, num_idxs_reg=ni, elem_size=ROW,
            single_packet=False, queue_num=next(swq) % 4)

    with tile.TileContext(nc, num_cores=NCORES) as tc:
        with (
            tc.tile_pool(name="const", bufs=1) as cpool,
            tc.tile_pool(name="rt", bufs=1) as rtpool,
            tc.tile_pool(name="work", bufs=2) as wp,
            tc.tile_pool(name="small", bufs=3) as sp,
            tc.tile_pool(name="psA", bufs=2, space="PSUM") as psA,   # agg
            tc.tile_pool(name="psD", bufs=2, space="PSUM") as psD,   # dense
            tc.tile_pool(name="psT", bufs=2, space="PSUM") as psT,   # transposes
            tc.tile_pool(name="psE", bufs=2, space="PSUM") as psE,   # el expand
        ):
            # ---------- constants ----------
            b1sb = cpool.tile([128, NH], F32)
            nc.sync.dma_start(out=b1sb[:], in_=d_b1.ap())
            b2sb = cpool.tile([128, C], F32)
            nc.sync.dma_start(out=b2sb[:], in_=d_b2.ap())
            ident = cpool.tile([128, 128], F16)
            make_identity(nc, ident[:])
            W1cat = cpool.tile([128, 2, 48], F16)
            nc.sync.dma_start(out=W1cat[:], in_=d_W1.ap())
            W2cat = cpool.tile([128, 2, 49], F16)
            nc.sync.dma_start(out=W2cat[:], in_=d_W2.ap())
            sidx_sb = cpool.tile([128, sidx_cols], I16)
            nc.sync.dma_start(out=sidx_sb[:], in_=d_sidx.ap())

            rT = rtpool.tile([128, 2, NPAD], F16)   # transposed post-elu l1 out
            el2sb = rtpool.tile([128, T_], F32)     # own layer-2 el, per tile

            # ---------- phase D1 (replicated): h/er table for ALL nodes ----
            G4 = 4  # node tiles per group
            ngroups = (TG + G4 - 1) // G4
            for g in range(ngroups):
                tiles = min(G4, TG - g * G4)
                ncols = tiles * 128
                xa = wp.tile([128, 2, G4 * 128], F16, tag="xa")
                nc.sync.dma_start(
                    out=xa[:, :, 0:ncols],
                    in_=d_xTf.ap()[:, :, g * G4 * 128:g * G4 * 128 + ncols]
                    .rearrange("q p n -> p q n"))
                h4 = wp.tile([128, G4, ROW], F32, tag="h4")
                for j in range(tiles):
                    ps = psD.tile([128, 48], F32, tag="dense")
                    for q in range(2):
                        nc.tensor.matmul(out=ps[:],
                                         lhsT=xa[:, q, j * 128:(j + 1) * 128],
                                         rhs=W1cat[:, q, :],
                                         start=q == 0, stop=q == 1)
                    nc.scalar.copy(out=h4[:, j, 0:48], in_=ps[:])
                base = g * G4 * 128
                rows = min(N - base, ncols)
                full_j = rows // 128
                if full_j:
                    nc.sync.dma_start(
                        out=d_t1.ap()[base:base + full_j * 128, 0:48]
                        .rearrange("(j p) f -> p j f", p=128),
                        in_=h4[:, 0:full_j, 0:48])
                rem = rows - full_j * 128
                if rem:
                    nc.sync.dma_start(
                        out=d_t1.ap()[base + full_j * 128:base + rows, 0:48],
                        in_=h4[0:rem, full_j, 0:48])
            tc.strict_bb_all_engine_barrier()

            # ---------- phase S1 (+ fused D2) ----------
            for t in range(T_):
                rows = 128 if t < T_ - 1 else LAST_ROWS
                ck = cpt[t]

                G1 = wp.tile([128, cptmax, ROW], F32, tag="G1")
                if ablate_gather:
                    nc.sync.dma_start(
                        out=G1[:, 0:ck, :], in_=d_t1.ap()[0:128, :]
                        .rearrange("p f -> p () f").to_broadcast([128, ck, ROW]))
                else:
                    gather(G1[:, 0:cA[t], :], d_t1.ap(), t, 0)
                    gather(G1[:, cA[t]:ck, :], d_t1.ap()[HALF:N, :], t, 1)

                # own el for this tile: el = x @ (W1 @ Wl1)  (cols 40:48)
                xo = sp.tile([128, 2, 128], F16, tag="xo")
                nc.sync.dma_start(
                    out=xo[:],
                    in_=d_xTo.ap()[:, :, t * 128:(t + 1) * 128]
                    .rearrange("q p n -> p q n"))
                pel = psD.tile([128, 8], F32, tag="dense")
                for q in range(2):
                    nc.tensor.matmul(out=pel[:], lhsT=xo[:, q, :],
                                     rhs=W1cat[:, q, 40:48],
                                     start=q == 0, stop=q == 1)
                elt = sp.tile([128, H], F16, tag="elt")
                nc.vector.tensor_copy(out=elt[:], in_=pel[:])

                # fp8 masks: [mskT(ck*128) | msk(ck*128)] in one load
                mts = wp.tile([128, cptmax * 256], F8, tag="mts")
                nc.sync.dma_start(
                    out=mts[:, 0:ck * 256],
                    in_=d_masks.ap()[:, off[t] * 256:off[t] * 256 + ck * 256])
                pse = psE.tile([128, cptmax * H], F32, tag="elexp")
                for k in range(ck):
                    nc.tensor.matmul(out=pse[:, k * H:(k + 1) * H],
                                     lhsT=mts[:, k * 128:(k + 1) * 128],
                                     rhs=elt[:],
                                     start=k == 0, stop=k == ck - 1,
                                     skip_group_check=True)

                # s = er[dst] + el[src]; leaky; exp
                s = wp.tile([128, cptmax, H], F32, tag="s")
                nc.vector.tensor_tensor(
                    out=s[:, 0:ck, :], in0=G1[:, 0:ck, 32:40],
                    in1=pse[:, 0:ck * H].rearrange("p (k h) -> p k h", h=H),
                    op=ALU.add)
                sL = wp.tile([128, cptmax, H], F32, tag="sL")
                nc.vector.scalar_tensor_tensor(
                    out=sL[:, 0:ck, :], in0=s[:, 0:ck, :], scalar=0.2,
                    in1=s[:, 0:ck, :], op0=ALU.mult, op1=ALU.max)
                rhs = wp.tile([128, cptmax, 264], F16, tag="rhs")
                nc.scalar.activation(out=rhs[:, 0:ck, 256:264],
                                     in_=sL[:, 0:ck, :], func=ACT.Exp)

                hdb = wp.tile([128, cptmax, F1], F16, tag="hdb")
                nc.scalar.copy(out=hdb[:, 0:ck, :], in_=G1[:, 0:ck, 0:32])

                nc.vector.tensor_tensor(
                    out=rhs[:, 0:ck, 0:256].rearrange(
                        "p k (h f) -> p k h f", h=H),
                    in0=rhs[:, 0:ck, 256:264].rearrange(
                        "p k (h o) -> p k h o", o=1)
                    .to_broadcast([128, ck, H, F1]),
                    in1=hdb[:, 0:ck, :].rearrange(
                        "p k (o f) -> p k o f", o=1)
                    .to_broadcast([128, ck, H, F1]),
                    op=ALU.mult)

                ps1 = psA.tile([128, 264], F32, tag="agg")
                for k in range(ck):
                    nc.tensor.matmul(out=ps1[:],
                                     lhsT=mts[:, (ck + k) * 128:(ck + k + 1) * 128],
                                     rhs=rhs[:, k, :],
                                     start=k == 0, stop=k == ck - 1)

                # epilogue: out1 = agg/denom + b1 ; r = elu(out1); rT = r.T
                dn = sp.tile([128, H], F32, tag="dn")
                nc.vector.tensor_scalar(out=dn[:], in0=ps1[:, 256:264],
                                        scalar1=1e-12, scalar2=None, op0=ALU.max)
                rc = sp.tile([128, H], F32, tag="rc")
                nc.vector.reciprocal(out=rc[:], in_=dn[:])
                o1 = wp.tile([128, NH], F32, tag="o1")
                nc.vector.tensor_tensor(
                    out=o1[:].rearrange("p (h f) -> p h f", h=H),
                    in0=ps1[:, 0:256].rearrange("p (h f) -> p h f", h=H),
                    in1=rc[:].rearrange("p (h o) -> p h o", o=1)
                    .to_broadcast([128, H, F1]),
                    op=ALU.mult)
                o1b = wp.tile([128, NH], F32, tag="o1b")
                nc.vector.tensor_tensor(out=o1b[:], in0=o1[:], in1=b1sb[:],
                                        op=ALU.add)
                # elu(x) = max(x,0)-1 + min(exp(x),1)
                ex = wp.tile([128, NH], F32, tag="ex")
                nc.scalar.activation(out=ex[:], in_=o1b[:], func=ACT.Exp)
                p1 = wp.tile([128, NH], F32, tag="p1")
                nc.vector.tensor_scalar(out=p1[:], in0=o1b[:], scalar1=0.0,
                                        scalar2=-1.0, op0=ALU.max, op1=ALU.add)
                r_ = wp.tile([128, NH], F16, tag="r_")
                nc.vector.scalar_tensor_tensor(out=r_[:], in0=ex[:], scalar=1.0,
                                               in1=p1[:], op0=ALU.min,
                                               op1=ALU.add)
                for q in range(2):
                    pt = psT.tile([128, 128], F16, tag="pt")
                    nc.tensor.transpose(out=pt[:], in_=r_[:, q * 128:(q + 1) * 128],
                                        identity=ident[:])
                    if q == 0:
                        nc.scalar.copy(out=rT[:, q, t * 128:(t + 1) * 128],
                                       in_=pt[:])
                    else:
                        nc.vector.tensor_copy(out=rT[:, q, t * 128:(t + 1) * 128],
                                              in_=pt[:])

                # D2: h2/er2/el2 for this tile
                ps2 = psD.tile([128, 49], F32, tag="dense")
                for q in range(2):
                    nc.tensor.matmul(out=ps2[:], lhsT=rT[:, q, t * 128:(t + 1) * 128],
                                     rhs=W2cat[:, q, :], start=q == 0, stop=q == 1)
                h2sb = wp.tile([128, ROW], F32, tag="h2sb")
                nc.scalar.copy(out=h2sb[:, 0:48], in_=ps2[:, 0:48])
                nc.scalar.copy(out=el2sb[:, t:t + 1], in_=ps2[:, 48:49])
                nc.sync.dma_start(out=d_t2loc.ap()[t * 128:t * 128 + rows, 0:48],
                                  in_=h2sb[0:rows, 0:48])

            # ---------- C2: share layer-2 table ----------
            if ablate_coll:
                nc.sync.dma_start(out=d_t2.ap()[0:NPC, :], in_=d_t2loc.ap())
            else:
                nc.gpsimd.collective_compute(
                    "AllGather", ALU.bypass, replica_groups=groups,
                    ins=[d_t2loc.ap()], outs=[d_t2.ap()])
            tc.strict_bb_all_engine_barrier()

            # ---------- phase S2 ----------
            for t in range(T_):
                rows = 128 if t < T_ - 1 else LAST_ROWS
                ck = cpt[t]

                Gt = wp.tile([128, cptmax, ROW], F32, tag="Gt")
                if ablate_gather:
                    nc.sync.dma_start(
                        out=Gt[:, 0:ck, :], in_=d_t2.ap()[0:128, :]
                        .rearrange("p f -> p () f").to_broadcast([128, ck, ROW]))
                else:
                    gather(Gt[:, 0:cA[t], :], d_t2.ap(), t, 0)
                    gather(Gt[:, cA[t]:ck, :], d_t2.ap()[HALF:N, :], t, 1)

                mts = wp.tile([128, cptmax * 256], F8, tag="mts")
                nc.sync.dma_start(
                    out=mts[:, 0:ck * 256],
                    in_=d_masks.ap()[:, off[t] * 256:off[t] * 256 + ck * 256])
                el2t = sp.tile([128, 1], F16, tag="el2t")
                nc.vector.tensor_copy(out=el2t[:], in_=el2sb[:, t:t + 1])
                pse2 = psE.tile([128, cptmax], F32, tag="elexp")
                for k in range(ck):
                    nc.tensor.matmul(out=pse2[:, k:k + 1],
                                     lhsT=mts[:, k * 128:(k + 1) * 128],
                                     rhs=el2t[:],
                                     start=k == 0, stop=k == ck - 1,
                                     skip_group_check=True)

                s2 = sp.tile([128, cptmax], F32, tag="s2")
                nc.vector.tensor_tensor(
                    out=s2[:, 0:ck],
                    in0=Gt[:, 0:ck, 47:48].rearrange("p k o -> p (k o)"),
                    in1=pse2[:, 0:ck], op=ALU.add)
                sL2 = sp.tile([128, cptmax], F32, tag="sL2")
                nc.vector.scalar_tensor_tensor(
                    out=sL2[:, 0:ck], in0=s2[:, 0:ck], scalar=0.2,
                    in1=s2[:, 0:ck], op0=ALU.mult, op1=ALU.max)
                rhs2 = wp.tile([128, cptmax, 48], F16, tag="rhs2")
                nc.scalar.activation(
                    out=rhs2[:, 0:ck, 47:48].rearrange("p k o -> p (k o)"),
                    in_=sL2[:, 0:ck], func=ACT.Exp)

                nc.vector.tensor_tensor(
                    out=rhs2[:, 0:ck, 0:47], in0=Gt[:, 0:ck, 0:47],
                    in1=rhs2[:, 0:ck, 47:48].to_broadcast([128, ck, C]),
                    op=ALU.mult)
                ps3 = psA.tile([128, 48], F32, tag="agg")
                for k in range(ck):
                    nc.tensor.matmul(out=ps3[:],
                                     lhsT=mts[:, (ck + k) * 128:(ck + k + 1) * 128],
                                     rhs=rhs2[:, k, :],
                                     start=k == 0, stop=k == ck - 1)

                # epilogue: out2 = agg2/denom2 + b2, then log_softmax
                dn2 = sp.tile([128, 1], F32, tag="dn2")
                nc.vector.tensor_scalar(out=dn2[:], in0=ps3[:, 47:48],
                                        scalar1=1e-12, scalar2=None, op0=ALU.max)
                rc2 = sp.tile([128, 1], F32, tag="rc2")
                nc.vector.reciprocal(out=rc2[:], in_=dn2[:])
                o2b = wp.tile([128, C], F32, tag="o2b")
                nc.vector.scalar_tensor_tensor(out=o2b[:], in0=ps3[:, 0:47],
                                               scalar=rc2[:, 0:1], in1=b2sb[:],
                                               op0=ALU.mult, op1=ALU.add)
                mx = sp.tile([128, 1], F32, tag="mx")
                nc.vector.tensor_reduce(out=mx[:], in_=o2b[:],
                                        axis=mybir.AxisListType.X, op=ALU.max)
                xm = wp.tile([128, C], F32, tag="xm")
                nc.vector.tensor_scalar(out=xm[:], in0=o2b[:], scalar1=mx[:, 0:1],
                                        scalar2=None, op0=ALU.subtract)
                ex2 = wp.tile([128, C], F32, tag="ex2")
                se = sp.tile([128, 1], F32, tag="se")
                nc.scalar.activation(out=ex2[:], in_=xm[:], func=ACT.Exp,
                                     accum_out=se[:])
                ls = sp.tile([128, 1], F32, tag="ls")
                nc.scalar.activation(out=ls[:], in_=se[:], func=ACT.Ln)
                fin = wp.tile([128, C], F32, tag="fin")
                nc.vector.tensor_scalar(out=fin[:], in0=xm[:], scalar1=ls[:, 0:1],
                                        scalar2=None, op0=ALU.subtract)
                nc.sync.dma_start(out=d_out.ap()[t * 128:t * 128 + rows, :],
                                  in_=fin[0:rows, :])

    nc.compile()
    return nc


def _make_inputs(x, edge_src, edge_dst, W1, Wl1, Wr1, b1, W2, Wl2, Wr2, b2):
    edge_ins, cA, cB = _preprocess(edge_src, edge_dst)
    x = np.asarray(x, dtype=np.float32)
    W1 = np.asarray(W1, dtype=np.float32)
    Wl1 = np.asarray(Wl1, dtype=np.float32)
    Wr1 = np.asarray(Wr1, dtype=np.float32)
    W2 = np.asarray(W2, dtype=np.float32)
    Wl2 = np.asarray(Wl2, dtype=np.float32)
    Wr2 = np.asarray(Wr2, dtype=np.float32)

    W1cat = np.zeros((128, 2, 48), dtype=np.float16)
    W1cat[:, :, 0:32] = W1.reshape(2, 128, F1).transpose(1, 0, 2)
    W1cat[:, :, 32:40] = (W1 @ Wr1).reshape(2, 128, H).transpose(1, 0, 2)
    W1cat[:, :, 40:48] = (W1 @ Wl1).reshape(2, 128, H).transpose(1, 0, 2)
    W2cat = np.zeros((128, 2, 49), dtype=np.float16)
    W2cat[:, :, 0:47] = W2.reshape(2, 128, C).transpose(1, 0, 2)
    W2cat[:, :, 47:48] = (W2 @ Wr2).reshape(2, 128, 1).transpose(1, 0, 2)
    W2cat[:, :, 48:49] = (W2 @ Wl2).reshape(2, 128, 1).transpose(1, 0, 2)

    b1f = np.tile(np.tile(np.asarray(b1, np.float32), H)[None, :], (128, 1))
    b2f = np.tile(np.asarray(b2, np.float32)[None, :], (128, 1))

    xTf = np.zeros((2, 128, NGPAD), dtype=np.float16)
    xTf[:, :, :N] = np.ascontiguousarray(x.T).reshape(2, 128, N)

    common = {
        "xTf": xTf, "W1cat": W1cat, "W2cat": W2cat,
        "b1f": b1f, "b2f": b2f,
    }
    in_maps = []
    for c in range(NCORES):
        xTo = np.zeros((2, 128, NPAD), dtype=np.float16)
        xs = np.ascontiguousarray(x[c * NPC:(c + 1) * NPC].T)   # [256, NPC]
        xTo[:, :, :NPC] = xs.reshape(2, 128, NPC)
        m = dict(common)
        m["xTo"] = xTo
        m.update(edge_ins[c])
        in_maps.append(m)
    return in_maps, (cA, cB)


def _run(inputs, trace=False, **build_kw):
    in_maps, key = _make_inputs(**inputs)
    ck = (key, tuple(sorted(build_kw.items())))
    if ck not in _cache:
        _cache[ck] = _build(*key, **build_kw)
    nc = _cache[ck]
    bkr = run_bass_kernel_spmd(nc, in_maps, list(range(NCORES)), trace=trace)
    out = np.concatenate([bkr.results[c]["out"] for c in range(NCORES)], axis=0)
    return out.astype(np.float32), bkr


def kernel(**inputs):
    out, _ = _run(inputs, trace=False)
    return out


# revision 38
# speedup vs baseline: 1.3163x; 1.0414x over previous
"""GAT (2-layer graph attention network) on 8 Trainium2 NeuronCores.

Strategy: node partition. Core c owns nodes [c*6250, (c+1)*6250) and all edges
whose src lies in its range (segment sums in the reference are over src).

Layer-1 node table (h|er) is computed REPLICATED on every core (the dense
matmul for all 50000 nodes is ~16us of PE time), which removes the first
AllGather entirely. The layer-2 table still needs an AllGather (it depends on
the sparse phase output).

Sparse phase: per 128-src-node tile, edges packed into 128-row chunks. h[dst]
rows are fetched with dma_gather (2 per tile: table split in two 25000-row
halves so indices fit int16), attention coefficients via mask matmuls
(one-hot src masks), segment sums as mask-matmuls accumulated in PSUM.

Self-contained: only needs numpy + the concourse (Bass) stack at
/opt/trn_rl_repo. All shapes hardcoded for the nn_GAT problem.
"""
import sys

if "/opt/trn_rl_repo" not in sys.path:
    sys.path.insert(0, "/opt/trn_rl_repo")

import numpy as np

import concourse.bacc as bacc
import concourse.bass as bass
import concourse.mybir as mybir
import concourse.tile as tile
from concourse.bass_utils import run_bass_kernel_spmd
from concourse.masks import make_identity

# problem shapes
N = 50000
E = 800000
FIN = 256
H = 8          # heads, layer 1
F1 = 32        # per-head features, layer 1
NH = 256       # hidden = H*F1
C = 47         # classes
NCORES = 8
NPC = N // NCORES          # nodes per core = 6250
T = (NPC + 127) // 128     # src tiles per core = 49
LAST_ROWS = NPC - (T - 1) * 128   # rows in last tile = 106
NPAD = T * 128             # padded node count per core = 6272
TG = (N + 127) // 128      # global node tiles = 391
LASTG = N - (TG - 1) * 128  # rows in last global tile = 80
NGPAD = TG * 128           # padded global node count = 50048
HALF = 25000               # table split point (int16 gather indices)
ROW = 64                   # table row f32 elements (256B, dma_gather granule)

F32 = mybir.dt.float32
F16 = mybir.dt.float16
F8 = mybir.dt.float8e4
I16 = mybir.dt.int16

ALU = mybir.AluOpType
ACT = mybir.ActivationFunctionType

_cache = {}


PREP_FP8_MASKS = False  # fp8 HBM-streamed masks lost the A/B to on-chip builds


def _preprocess(edge_src, edge_dst):
    """Sort/group/pad edges; build per-core index/mask arrays (numpy only).

    Per core, per src tile: edges split into A (dst < HALF) and B (dst >=
    HALF) sections, each padded to a multiple of 128 (chunk counts cA/cB are
    maxed across cores so all cores run the same program). Edge slot (p, k)
    <-> flat position k*128+p.
    """
    order = np.argsort(edge_src, kind="stable")
    src_s = np.asarray(edge_src)[order].astype(np.int64)
    dst_s = np.asarray(edge_dst)[order].astype(np.int64)
    bounds = np.searchsorted(src_s, np.arange(NCORES + 1) * NPC)

    # gather per-(core, tile) A/B edge lists
    AB = []  # [core][tile] -> (srcA, dstA, srcB, dstB)  (src local to tile)
    for c in range(NCORES):
        b0, b1 = bounds[c], bounds[c + 1]
        sc = src_s[b0:b1] - c * NPC
        dc = dst_s[b0:b1]
        tloc = sc >> 7
        rows = []
        for t in range(T):
            m = tloc == t
            st, dt_ = sc[m] - t * 128, dc[m]
            a = dt_ < HALF
            rows.append((st[a], dt_[a], st[~a], dt_[~a] - HALF))
        AB.append(rows)

    cA = [max(1, max((len(AB[c][t][0]) + 127) // 128 for c in range(NCORES)))
          for t in range(T)]
    cB = [max(1, max((len(AB[c][t][2]) + 127) // 128 for c in range(NCORES)))
          for t in range(T)]
    cpt = [cA[t] + cB[t] for t in range(T)]
    scols = sum(cpt)                    # total chunks
    sidx_cols = scols * 8               # int16 strip columns (128 idx -> 8)

    import ml_dtypes
    cpt_off = np.concatenate([[0], np.cumsum(cpt)]).astype(int)
    ins = []
    jj = np.arange(128)
    for c in range(NCORES):
        sidx = np.zeros((16, sidx_cols), dtype=np.int16)
        srcloc = np.full((128, scols), 1000, dtype=np.int32)
        srcflat = np.full((scols * 128,), 1000, dtype=np.int32)
        off = 0
        for t in range(T):
            sA, dA, sB, dB = AB[c][t]
            for (ss, dd, nk) in ((sA, dA, cA[t]), (sB, dB, cB[t])):
                ni = nk * 128
                dpad = np.zeros(ni, dtype=np.int64)
                dpad[:len(dd)] = dd
                # index strip: position i at (i%16, i//16)
                sidx[:, off * 8:(off + nk) * 8] = \
                    dpad.astype(np.int16).reshape(-1, 16).T
                i = np.arange(len(ss))
                srcloc[i % 128, off + i // 128] = ss
                srcflat[off * 128 + i] = ss
                off += nk
        entry = {
            "sidx": np.tile(sidx, (8, 1)),
            "srcloc2": srcloc.astype(np.float32),
            "srcflat2": srcflat.astype(np.float16)[None, :],
        }
        if PREP_FP8_MASKS:
            # fp8 one-hot masks (1.0 in e4m3 is 0x38):
            #   mskT[p, kj] = (src of edge slot kj == p)
            #   msk[p, k, j] = (srcloc[p, k] == j)
            mskT = (srcflat[None, :] == jj[:, None]).astype(np.uint8) * 0x38
            msk = (srcloc[:, :, None] == jj[None, None, :]).astype(np.uint8) * 0x38
            masks = np.zeros((128, scols * 256), dtype=np.uint8)
            for t in range(T):
                o, ck = cpt_off[t], cpt[t]
                base = o * 256
                masks[:, base:base + ck * 128] = mskT[:, o * 128:(o + ck) * 128]
                masks[:, base + ck * 128:base + ck * 256] = \
                    msk[:, o:o + ck, :].reshape(128, ck * 128)
            entry["masks"] = masks.view(ml_dtypes.float8_e4m3)
        ins.append(entry)
    return ins, tuple(cA), tuple(cB)


def _build(cA, cB, ablate_gather=False, ablate_coll=False, mask_mode="slb",
           no_barrier=False, d1split=False, use_lrelu=False):
    T_ = len(cA)
    cpt = [cA[t] + cB[t] for t in range(T_)]
    cptmax = max(cpt)
    scols = sum(cpt)
    sidx_cols = scols * 8
    # per-tile offsets into the concatenated strips
    off = np.concatenate([[0], np.cumsum(cpt)]).astype(int)

    nc = bacc.Bacc("TRN2", target_bir_lowering=False, debug=False,
                   num_devices=NCORES, num_swdge_queues=4)

    # ---- external inputs (per core) ----
    d_xTf = nc.dram_tensor("xTf", [2, 128, NGPAD], F16, kind="ExternalInput")
    d_xTo = nc.dram_tensor("xTo", [2, 128, NPAD], F16, kind="ExternalInput")
    d_W1 = nc.dram_tensor("W1cat", [128, 2, 48], F16, kind="ExternalInput")
    d_W2 = nc.dram_tensor("W2cat", [128, 2, 49], F16, kind="ExternalInput")
    d_b1 = nc.dram_tensor("b1f", [128, NH], F32, kind="ExternalInput")
    d_b2 = nc.dram_tensor("b2f", [128, C], F32, kind="ExternalInput")
    d_sidx = nc.dram_tensor("sidx", [128, sidx_cols], I16, kind="ExternalInput")
    if mask_mode == "fp8":
        d_masks = nc.dram_tensor("masks", [128, scols * 256], F8,
                                 kind="ExternalInput")
    else:
        d_srcloc = nc.dram_tensor("srcloc2", [128, scols], F32,
                                  kind="ExternalInput")
        d_srcflat = nc.dram_tensor("srcflat2", [1, scols * 128], F16,
                                   kind="ExternalInput")

    d_out = nc.dram_tensor("out", [NPC, C], F32, kind="ExternalOutput")

    # ---- internal DRAM tables ----
    d_t1 = nc.dram_tensor("t1", [N, ROW], F32)              # [h | er | pad]
    d_t2loc = nc.dram_tensor("t2loc", [NPC, ROW], F32)      # [h2 | er2 | pad]
    d_t2 = nc.dram_tensor("t2", [N, ROW], F32, addr_space="Shared")

    groups = [list(range(NCORES))]

    # SWDGE queue must track Tile's DMASW sem-lane round-robin (lane =
    # pool-DMA index % 8, queue = index % 4) or the sem gets updated from a
    # queue it isn't locked to.
    swq = iter(range(1 << 30))

    def gather(out_ap, table_ap, t, half):
        nk = (cA[t] if half == 0 else cB[t])
        o8 = (off[t] + (0 if half == 0 else cA[t])) * 8
        ni = nk * 128
        # single_packet=True wedges the SWDGE ucode above ~1024 idxs
        return nc.gpsimd.dma_gather(
            out_ap=out_ap, in_ap=table_ap,
            idxs_ap=sidx_sb[:, o8:o8 + nk * 8],
            num_idxs=ni, num_idxs_reg=ni, elem_size=ROW,
            single_packet=False, queue_num=next(swq) % 4)

    with tile.TileContext(nc, num_cores=NCORES) as tc:
        with (
            tc.tile_pool(name="const", bufs=1) as cpool,
            tc.tile_pool(name="rt", bufs=1) as rtpool,
            tc.tile_pool(name="work", bufs=2) as wp,
            tc.tile_pool(name="small", bufs=3) as sp,
            tc.tile_pool(name="psA", bufs=2, space="PSUM") as psA,   # agg
            tc.tile_pool(name="psD", bufs=2, space="PSUM") as psD,   # dense
            tc.tile_pool(name="psT", bufs=2, space="PSUM") as psT,   # transposes
            tc.tile_pool(name="psE", bufs=2, space="PSUM") as psE,   # el expand
        ):
            # ---------- constants ----------
            b1sb = cpool.tile([128, NH], F32)
            nc.sync.dma_start(out=b1sb[:], in_=d_b1.ap())
            b2sb = cpool.tile([128, C], F32)
            nc.sync.dma_start(out=b2sb[:], in_=d_b2.ap())
            ident = cpool.tile([128, 128], F16)
            make_identity(nc, ident[:])
            W1cat = cpool.tile([128, 2, 48], F16)
            nc.sync.dma_start(out=W1cat[:], in_=d_W1.ap())
            W2cat = cpool.tile([128, 2, 49], F16)
            nc.sync.dma_start(out=W2cat[:], in_=d_W2.ap())
            sidx_sb = cpool.tile([128, sidx_cols], I16)
            nc.sync.dma_start(out=sidx_sb[:], in_=d_sidx.ap())
            if mask_mode == "slb":
                iota = cpool.tile([128, 128], F16)
                nc.gpsimd.iota(out=iota[:], pattern=[[1, 128]], base=0,
                               channel_multiplier=0,
                               allow_small_or_imprecise_dtypes=True)
                iotac = cpool.tile([128, 1], F32)
                nc.gpsimd.iota(out=iotac[:], pattern=[[1, 1]], base=0,
                               channel_multiplier=1,
                               allow_small_or_imprecise_dtypes=True)
                srcl_sb = cpool.tile([128, scols], F32)
                nc.sync.dma_start(out=srcl_sb[:], in_=d_srcloc.ap())

            def load_masks(t, ck):
                """Returns (mskT_lhsT(k), msk_lhsT(k)) accessors for tile t."""
                if mask_mode == "fp8":
                    mts = wp.tile([128, cptmax * 256], F8, tag="mts")
                    nc.sync.dma_start(
                        out=mts[:, 0:ck * 256],
                        in_=d_masks.ap()[:, off[t] * 256:off[t] * 256 + ck * 256])
                    return (lambda k: mts[:, k * 128:(k + 1) * 128],
                            lambda k: mts[:, (ck + k) * 128:(ck + k + 1) * 128])
                slb = wp.tile([128, cptmax * 128], F16, tag="slb")
                nc.sync.dma_start(
                    out=slb[:, 0:ck * 128],
                    in_=d_srcflat.ap()[:, off[t] * 128:(off[t] + ck) * 128]
                    .to_broadcast([128, ck * 128]))
                mskT = wp.tile([128, cptmax * 128], F16, tag="mskT")
                nc.vector.tensor_scalar(out=mskT[:, 0:ck * 128],
                                        in0=slb[:, 0:ck * 128],
                                        scalar1=iotac[:],
                                        scalar2=None, op0=ALU.is_equal)
                msk = wp.tile([128, cptmax, 128], F16, tag="msk")
                nc.vector.tensor_tensor(
                    out=msk[:, 0:ck, :],
                    in0=srcl_sb[:, off[t]:off[t] + ck]
                    .rearrange("p k -> p k ()").to_broadcast([128, ck, 128]),
                    in1=iota[:].rearrange("p j -> p () j")
                    .to_broadcast([128, ck, 128]),
                    op=ALU.is_equal)
                return (lambda k: mskT[:, k * 128:(k + 1) * 128],
                        lambda k: msk[:, k, :])

            rT = rtpool.tile([128, 2, NPAD], F16)   # transposed post-elu l1 out
            el2sb = rtpool.tile([128, T_], F32)     # own layer-2 el, per tile

            # ---------- phase D1 (replicated): h/er table for ALL nodes ----
            G4 = 4  # node tiles per group
            ngroups = (TG + G4 - 1) // G4
            for g in range(ngroups):
                tiles = min(G4, TG - g * G4)
                ncols = tiles * 128
                xa = wp.tile([128, 2, G4 * 128], F16, tag="xa")
                nc.sync.dma_start(
                    out=xa[:, :, 0:ncols],
                    in_=d_xTf.ap()[:, :, g * G4 * 128:g * G4 * 128 + ncols]
                    .rearrange("q p n -> p q n"))
                h4 = wp.tile([128, G4, ROW], F32, tag="h4")
                for j in range(tiles):
                    ps = psD.tile([128, 48], F32, tag="dense")
                    for q in range(2):
                        nc.tensor.matmul(out=ps[:],
                                         lhsT=xa[:, q, j * 128:(j + 1) * 128],
                                         rhs=W1cat[:, q, :],
                                         start=q == 0, stop=q == 1)
                    if d1split and j % 2 == 1:
                        nc.vector.tensor_copy(out=h4[:, j, 0:48], in_=ps[:])
                    else:
                        nc.scalar.copy(out=h4[:, j, 0:48], in_=ps[:])
                base = g * G4 * 128
                rows = min(N - base, ncols)
                full_j = rows // 128
                if full_j:
                    nc.sync.dma_start(
                        out=d_t1.ap()[base:base + full_j * 128, 0:48]
                        .rearrange("(j p) f -> p j f", p=128),
                        in_=h4[:, 0:full_j, 0:48])
                rem = rows - full_j * 128
                if rem:
                    nc.sync.dma_start(
                        out=d_t1.ap()[base + full_j * 128:base + rows, 0:48],
                        in_=h4[0:rem, full_j, 0:48])
            if not no_barrier:
                tc.strict_bb_all_engine_barrier()

            # ---------- phase S1 (+ fused D2) ----------
            for t in range(T_):
                rows = 128 if t < T_ - 1 else LAST_ROWS
                ck = cpt[t]

                G1 = wp.tile([128, cptmax, ROW], F32, tag="G1")
                if ablate_gather:
                    nc.sync.dma_start(
                        out=G1[:, 0:ck, :], in_=d_t1.ap()[0:128, :]
                        .rearrange("p f -> p () f").to_broadcast([128, ck, ROW]))
                else:
                    gather(G1[:, 0:cA[t], :], d_t1.ap(), t, 0)
                    gather(G1[:, cA[t]:ck, :], d_t1.ap()[HALF:N, :], t, 1)

                # own el for this tile: el = x @ (W1 @ Wl1)  (cols 40:48)
                xo = sp.tile([128, 2, 128], F16, tag="xo")
                nc.sync.dma_start(
                    out=xo[:],
                    in_=d_xTo.ap()[:, :, t * 128:(t + 1) * 128]
                    .rearrange("q p n -> p q n"))
                pel = psD.tile([128, 8], F32, tag="dense")
                for q in range(2):
                    nc.tensor.matmul(out=pel[:], lhsT=xo[:, q, :],
                                     rhs=W1cat[:, q, 40:48],
                                     start=q == 0, stop=q == 1)
                elt = sp.tile([128, H], F16, tag="elt")
                nc.vector.tensor_copy(out=elt[:], in_=pel[:])

                mskT_at, msk_at = load_masks(t, ck)
                pse = psE.tile([128, cptmax * H], F32, tag="elexp")
                for k in range(ck):
                    nc.tensor.matmul(out=pse[:, k * H:(k + 1) * H],
                                     lhsT=mskT_at(k), rhs=elt[:],
                                     start=k == 0, stop=k == ck - 1,
                                     skip_group_check=True)

                # s = er[dst] + el[src]; leaky; exp
                s = wp.tile([128, cptmax, H], F32, tag="s")
                nc.vector.tensor_tensor(
                    out=s[:, 0:ck, :], in0=G1[:, 0:ck, 32:40],
                    in1=pse[:, 0:ck * H].rearrange("p (k h) -> p k h", h=H),
                    op=ALU.add)
                sL = wp.tile([128, cptmax, H], F32, tag="sL")
                if use_lrelu:
                    nc.scalar.activation(out=sL[:, 0:ck, :], in_=s[:, 0:ck, :],
                                         func=ACT.Lrelu, alpha=0.2)
                else:
                    nc.vector.scalar_tensor_tensor(
                        out=sL[:, 0:ck, :], in0=s[:, 0:ck, :], scalar=0.2,
                        in1=s[:, 0:ck, :], op0=ALU.mult, op1=ALU.max)
                rhs = wp.tile([128, cptmax, 264], F16, tag="rhs")
                nc.scalar.activation(out=rhs[:, 0:ck, 256:264],
                                     in_=sL[:, 0:ck, :], func=ACT.Exp)

                hdb = wp.tile([128, cptmax, F1], F16, tag="hdb")
                nc.scalar.copy(out=hdb[:, 0:ck, :], in_=G1[:, 0:ck, 0:32])

                nc.vector.tensor_tensor(
                    out=rhs[:, 0:ck, 0:256].rearrange(
                        "p k (h f) -> p k h f", h=H),
                    in0=rhs[:, 0:ck, 256:264].rearrange(
                        "p k (h o) -> p k h o", o=1)
                    .to_broadcast([128, ck, H, F1]),
                    in1=hdb[:, 0:ck, :].rearrange(
                        "p k (o f) -> p k o f", o=1)
                    .to_broadcast([128, ck, H, F1]),
                    op=ALU.mult)

                ps1 = psA.tile([128, 264], F32, tag="agg")
                for k in range(ck):
                    nc.tensor.matmul(out=ps1[:], lhsT=msk_at(k),
                                     rhs=rhs[:, k, :],
                                     start=k == 0, stop=k == ck - 1)

                # epilogue: out1 = agg/denom + b1 ; r = elu(out1); rT = r.T
                dn = sp.tile([128, H], F32, tag="dn")
                nc.vector.tensor_scalar(out=dn[:], in0=ps1[:, 256:264],
                                        scalar1=1e-12, scalar2=None, op0=ALU.max)
                rc = sp.tile([128, H], F32, tag="rc")
                nc.vector.reciprocal(out=rc[:], in_=dn[:])
                o1 = wp.tile([128, NH], F32, tag="o1")
                nc.vector.tensor_tensor(
                    out=o1[:].rearrange("p (h f) -> p h f", h=H),
                    in0=ps1[:, 0:256].rearrange("p (h f) -> p h f", h=H),
                    in1=rc[:].rearrange("p (h o) -> p h o", o=1)
                    .to_broadcast([128, H, F1]),
                    op=ALU.mult)
                o1b = wp.tile([128, NH], F32, tag="o1b")
                nc.vector.tensor_tensor(out=o1b[:], in0=o1[:], in1=b1sb[:],
                                        op=ALU.add)
                # elu(x) = max(x,0)-1 + min(exp(x),1)
                ex = wp.tile([128, NH], F32, tag="ex")
                nc.scalar.activation(out=ex[:], in_=o1b[:], func=ACT.Exp)
                p1 = wp.tile([128, NH], F32, tag="p1")
                nc.vector.tensor_scalar(out=p1[:], in0=o1b[:], scalar1=0.0,
                                        scalar2=-1.0, op0=ALU.max, op1=ALU.add)
                r_ = wp.tile([128, NH], F16, tag="r_")
                nc.vector.scalar_tensor_tensor(out=r_[:], in0=ex[:], scalar=1.0,
                                               in1=p1[:], op0=ALU.min,
                                               op1=ALU.add)
                for q in range(2):
                    pt = psT.tile([128, 128], F16, tag="pt")
                    nc.tensor.transpose(out=pt[:], in_=r_[:, q * 128:(q + 1) * 128],
                                        identity=ident[:])
                    if q == 0:
                        nc.scalar.copy(out=rT[:, q, t * 128:(t + 1) * 128],
                                       in_=pt[:])
                    else:
                        nc.vector.tensor_copy(out=rT[:, q, t * 128:(t + 1) * 128],
                                              in_=pt[:])

                # D2: h2/er2/el2 for this tile
                ps2 = psD.tile([128, 49], F32, tag="dense")
                for q in range(2):
                    nc.tensor.matmul(out=ps2[:], lhsT=rT[:, q, t * 128:(t + 1) * 128],
                                     rhs=W2cat[:, q, :], start=q == 0, stop=q == 1)
                h2sb = wp.tile([128, ROW], F32, tag="h2sb")
                nc.scalar.copy(out=h2sb[:, 0:48], in_=ps2[:, 0:48])
                nc.scalar.copy(out=el2sb[:, t:t + 1], in_=ps2[:, 48:49])
                nc.sync.dma_start(out=d_t2loc.ap()[t * 128:t * 128 + rows, 0:48],
                                  in_=h2sb[0:rows, 0:48])

            # ---------- C2: share layer-2 table ----------
            if ablate_coll:
                nc.sync.dma_start(out=d_t2.ap()[0:NPC, :], in_=d_t2loc.ap())
            else:
                nc.gpsimd.collective_compute(
                    "AllGather", ALU.bypass, replica_groups=groups,
                    ins=[d_t2loc.ap()], outs=[d_t2.ap()])
            if not no_barrier:
                tc.strict_bb_all_engine_barrier()

            # ---------- phase S2 ----------
            for t in range(T_):
                rows = 128 if t < T_ - 1 else LAST_ROWS
                ck = cpt[t]

                Gt = wp.tile([128, cptmax, ROW], F32, tag="Gt")
                if ablate_gather:
                    nc.sync.dma_start(
                        out=Gt[:, 0:ck, :], in_=d_t2.ap()[0:128, :]
                        .rearrange("p f -> p () f").to_broadcast([128, ck, ROW]))
                else:
                    gather(Gt[:, 0:cA[t], :], d_t2.ap(), t, 0)
                    gather(Gt[:, cA[t]:ck, :], d_t2.ap()[HALF:N, :], t, 1)

                mskT_at, msk_at = load_masks(t, ck)
                el2t = sp.tile([128, 1], F16, tag="el2t")
                nc.vector.tensor_copy(out=el2t[:], in_=el2sb[:, t:t + 1])
                pse2 = psE.tile([128, cptmax], F32, tag="elexp")
                for k in range(ck):
                    nc.tensor.matmul(out=pse2[:, k:k + 1],
                                     lhsT=mskT_at(k), rhs=el2t[:],
                                     start=k == 0, stop=k == ck - 1,
                                     skip_group_check=True)

                s2 = sp.tile([128, cptmax], F32, tag="s2")
                nc.vector.tensor_tensor(
                    out=s2[:, 0:ck],
                    in0=Gt[:, 0:ck, 47:48].rearrange("p k o -> p (k o)"),
                    in1=pse2[:, 0:ck], op=ALU.add)
                sL2 = sp.tile([128, cptmax], F32, tag="sL2")
                if use_lrelu:
                    nc.scalar.activation(out=sL2[:, 0:ck], in_=s2[:, 0:ck],
                                         func=ACT.Lrelu, alpha=0.2)
                else:
                    nc.vector.scalar_tensor_tensor(
                        out=sL2[:, 0:ck], in0=s2[:, 0:ck], scalar=0.2,
                        in1=s2[:, 0:ck], op0=ALU.mult, op1=ALU.max)
                rhs2 = wp.tile([128, cptmax, 48], F16, tag="rhs2")
                nc.scalar.activation(
                    out=rhs2[:, 0:ck, 47:48].rearrange("p k o -> p (k o)"),
                    in_=sL2[:, 0:ck], func=ACT.Exp)

                nc.vector.tensor_tensor(
                    out=rhs2[:, 0:ck, 0:47], in0=Gt[:, 0:ck, 0:47],
                    in1=rhs2[:, 0:ck, 47:48].to_broadcast([128, ck, C]),
                    op=ALU.mult)
                ps3 = psA.tile([128, 48], F32, tag="agg")
                for k in range(ck):
                    nc.tensor.matmul(out=ps3[:], lhsT=msk_at(k),
                                     rhs=rhs2[:, k, :],
                                     start=k == 0, stop=k == ck - 1)

                # epilogue: out2 = agg2/denom2 + b2, then log_softmax
                dn2 = sp.tile([128, 1], F32, tag="dn2")
                nc.vector.tensor_scalar(out=dn2[:], in0=ps3[:, 47:48],
                                        scalar1=1e-12, scalar2=None, op0=ALU.max)
                rc2 = sp.tile([128, 1], F32, tag="rc2")
                nc.vector.reciprocal(out=rc2[:], in_=dn2[:])
                o2b = wp.tile([128, C], F32, tag="o2b")
                nc.vector.scalar_tensor_tensor(out=o2b[:], in0=ps3[:, 0:47],
                                               scalar=rc2[:, 0:1], in1=b2sb[:],
                                               op0=ALU.mult, op1=ALU.add)
                mx = sp.tile([128, 1], F32, tag="mx")
                nc.vector.tensor_reduce(out=mx[:], in_=o2b[:],
                                        axis=mybir.AxisListType.X, op=ALU.max)
                xm = wp.tile([128, C], F32, tag="xm")
                nc.vector.tensor_scalar(out=xm[:], in0=o2b[:], scalar1=mx[:, 0:1],
                                        scalar2=None, op0=ALU.subtract)
                ex2 = wp.tile([128, C], F32, tag="ex2")
                se = sp.tile([128, 1], F32, tag="se")
                nc.scalar.activation(out=ex2[:], in_=xm[:], func=ACT.Exp,
                                     accum_out=se[:])
                ls = sp.tile([128, 1], F32, tag="ls")
                nc.scalar.activation(out=ls[:], in_=se[:], func=ACT.Ln)
                fin = wp.tile([128, C], F32, tag="fin")
                nc.vector.tensor_scalar(out=fin[:], in0=xm[:], scalar1=ls[:, 0:1],
                                        scalar2=None, op0=ALU.subtract)
                nc.sync.dma_start(out=d_out.ap()[t * 128:t * 128 + rows, :],
                                  in_=fin[0:rows, :])

    nc.compile()
    return nc


def _make_inputs(x, edge_src, edge_dst, W1, Wl1, Wr1, b1, W2, Wl2, Wr2, b2):
    edge_ins, cA, cB = _preprocess(edge_src, edge_dst)
    x = np.asarray(x, dtype=np.float32)
    W1 = np.asarray(W1, dtype=np.float32)
    Wl1 = np.asarray(Wl1, dtype=np.float32)
    Wr1 = np.asarray(Wr1, dtype=np.float32)
    W2 = np.asarray(W2, dtype=np.float32)
    Wl2 = np.asarray(Wl2, dtype=np.float32)
    Wr2 = np.asarray(Wr2, dtype=np.float32)

    W1cat = np.zeros((128, 2, 48), dtype=np.float16)
    W1cat[:, :, 0:32] = W1.reshape(2, 128, F1).transpose(1, 0, 2)
    W1cat[:, :, 32:40] = (W1 @ Wr1).reshape(2, 128, H).transpose(1, 0, 2)
    W1cat[:, :, 40:48] = (W1 @ Wl1).reshape(2, 128, H).transpose(1, 0, 2)
    W2cat = np.zeros((128, 2, 49), dtype=np.float16)
    W2cat[:, :, 0:47] = W2.reshape(2, 128, C).transpose(1, 0, 2)
    W2cat[:, :, 47:48] = (W2 @ Wr2).reshape(2, 128, 1).transpose(1, 0, 2)
    W2cat[:, :, 48:49] = (W2 @ Wl2).reshape(2, 128, 1).transpose(1, 0, 2)

    b1f = np.tile(np.tile(np.asarray(b1, np.float32), H)[None, :], (128, 1))
    b2f = np.tile(np.asarray(b2, np.float32)[None, :], (128, 1))

    xTf = np.zeros((2, 128, NGPAD), dtype=np.float16)
    xTf[:, :, :N] = np.ascontiguousarray(x.T).reshape(2, 128, N)

    common = {
        "xTf": xTf, "W1cat": W1cat, "W2cat": W2cat,
        "b1f": b1f, "b2f": b2f,
    }
    in_maps = []
    for c in range(NCORES):
        xTo = np.zeros((2, 128, NPAD), dtype=np.float16)
        xs = np.ascontiguousarray(x[c * NPC:(c + 1) * NPC].T)   # [256, NPC]
        xTo[:, :, :NPC] = xs.reshape(2, 128, NPC)
        m = dict(common)
        m["xTo"] = xTo
        m.update(edge_ins[c])
        in_maps.append(m)
    return in_maps, (cA, cB)


def _run(inputs, trace=False, **build_kw):
    in_maps, key = _make_inputs(**inputs)
    ck = (key, tuple(sorted(build_kw.items())))
    if ck not in _cache:
        _cache[ck] = _build(*key, **build_kw)
    nc = _cache[ck]
    bkr = run_bass_kernel_spmd(nc, in_maps, list(range(NCORES)), trace=trace)
    out = np.concatenate([bkr.results[c]["out"] for c in range(NCORES)], axis=0)
    return out.astype(np.float32), bkr


def kernel(**inputs):
    out, _ = _run(inputs, trace=False)
    return out


# revision 40
# speedup vs baseline: 1.4001x; 1.0637x over previous
"""GAT (2-layer graph attention network) on 8 Trainium2 NeuronCores.

Strategy: node partition. Core c owns nodes [c*6250, (c+1)*6250) and all edges
whose src lies in its range (segment sums in the reference are over src).

Layer-1 node table (h|er) is computed REPLICATED on every core (the dense
matmul for all 50000 nodes is ~16us of PE time), which removes the first
AllGather entirely. The layer-2 table still needs an AllGather (it depends on
the sparse phase output).

Sparse phase: per 128-src-node tile, edges packed into 128-row chunks. h[dst]
rows are fetched with dma_gather (2 per tile: table split in two 25000-row
halves so indices fit int16), attention coefficients via mask matmuls
(one-hot src masks), segment sums as mask-matmuls accumulated in PSUM.

Self-contained: only needs numpy + the concourse (Bass) stack at
/opt/trn_rl_repo. All shapes hardcoded for the nn_GAT problem.
"""
import sys

if "/opt/trn_rl_repo" not in sys.path:
    sys.path.insert(0, "/opt/trn_rl_repo")

import numpy as np

import concourse.bacc as bacc
import concourse.bass as bass
import concourse.mybir as mybir
import concourse.tile as tile
from concourse.bass_utils import run_bass_kernel_spmd
from concourse.masks import make_identity

# problem shapes
N = 50000
E = 800000
FIN = 256
H = 8          # heads, layer 1
F1 = 32        # per-head features, layer 1
NH = 256       # hidden = H*F1
C = 47         # classes
NCORES = 8
NPC = N // NCORES          # nodes per core = 6250
T = (NPC + 127) // 128     # src tiles per core = 49
LAST_ROWS = NPC - (T - 1) * 128   # rows in last tile = 106
NPAD = T * 128             # padded node count per core = 6272
TG = (N + 127) // 128      # global node tiles = 391
LASTG = N - (TG - 1) * 128  # rows in last global tile = 80
NGPAD = TG * 128           # padded global node count = 50048
HALF = 25000               # table split point (int16 gather indices)
ROW = 64                   # table row f32 elements (256B, dma_gather granule)

F32 = mybir.dt.float32
F16 = mybir.dt.float16
F8 = mybir.dt.float8e4
I16 = mybir.dt.int16

ALU = mybir.AluOpType
ACT = mybir.ActivationFunctionType

_cache = {}


PREP_FP8_MASKS = False  # fp8 HBM-streamed masks lost the A/B to on-chip builds


def _preprocess(edge_src, edge_dst):
    """Sort/group/pad edges; build per-core index/mask arrays (numpy only).

    Per core, per src tile: edges split into A (dst < HALF) and B (dst >=
    HALF) sections, each padded to a multiple of 128 (chunk counts cA/cB are
    maxed across cores so all cores run the same program). Edge slot (p, k)
    <-> flat position k*128+p.
    """
    order = np.argsort(edge_src, kind="stable")
    src_s = np.asarray(edge_src)[order].astype(np.int64)
    dst_s = np.asarray(edge_dst)[order].astype(np.int64)
    bounds = np.searchsorted(src_s, np.arange(NCORES + 1) * NPC)

    # gather per-(core, tile) A/B edge lists
    AB = []  # [core][tile] -> (srcA, dstA, srcB, dstB)  (src local to tile)
    for c in range(NCORES):
        b0, b1 = bounds[c], bounds[c + 1]
        sc = src_s[b0:b1] - c * NPC
        dc = dst_s[b0:b1]
        tloc = sc >> 7
        rows = []
        for t in range(T):
            m = tloc == t
            st, dt_ = sc[m] - t * 128, dc[m]
            a = dt_ < HALF
            rows.append((st[a], dt_[a], st[~a], dt_[~a] - HALF))
        AB.append(rows)

    cA = [max(1, max((len(AB[c][t][0]) + 127) // 128 for c in range(NCORES)))
          for t in range(T)]
    cB = [max(1, max((len(AB[c][t][2]) + 127) // 128 for c in range(NCORES)))
          for t in range(T)]
    cpt = [cA[t] + cB[t] for t in range(T)]
    scols = sum(cpt)                    # total chunks
    sidx_cols = scols * 8               # int16 strip columns (128 idx -> 8)

    import ml_dtypes
    cpt_off = np.concatenate([[0], np.cumsum(cpt)]).astype(int)
    ins = []
    jj = np.arange(128)
    for c in range(NCORES):
        sidx = np.zeros((16, sidx_cols), dtype=np.int16)
        srcloc = np.full((128, scols), 1000, dtype=np.int32)
        srcflat = np.full((scols * 128,), 1000, dtype=np.int32)
        off = 0
        for t in range(T):
            sA, dA, sB, dB = AB[c][t]
            for (ss, dd, nk) in ((sA, dA, cA[t]), (sB, dB, cB[t])):
                ni = nk * 128
                dpad = np.zeros(ni, dtype=np.int64)
                dpad[:len(dd)] = dd
                # index strip: position i at (i%16, i//16)
                sidx[:, off * 8:(off + nk) * 8] = \
                    dpad.astype(np.int16).reshape(-1, 16).T
                i = np.arange(len(ss))
                srcloc[i % 128, off + i // 128] = ss
                srcflat[off * 128 + i] = ss
                off += nk
        entry = {
            "sidx": np.tile(sidx, (8, 1)),
            "srcloc2": srcloc.astype(np.float32),
            "srcflat2": srcflat.astype(np.float16)[None, :],
        }
        if PREP_FP8_MASKS:
            # fp8 one-hot masks (1.0 in e4m3 is 0x38):
            #   mskT[p, kj] = (src of edge slot kj == p)
            #   msk[p, k, j] = (srcloc[p, k] == j)
            mskT = (srcflat[None, :] == jj[:, None]).astype(np.uint8) * 0x38
            msk = (srcloc[:, :, None] == jj[None, None, :]).astype(np.uint8) * 0x38
            masks = np.zeros((128, scols * 256), dtype=np.uint8)
            for t in range(T):
                o, ck = cpt_off[t], cpt[t]
                base = o * 256
                masks[:, base:base + ck * 128] = mskT[:, o * 128:(o + ck) * 128]
                masks[:, base + ck * 128:base + ck * 256] = \
                    msk[:, o:o + ck, :].reshape(128, ck * 128)
            entry["masks"] = masks.view(ml_dtypes.float8_e4m3)
        ins.append(entry)
    return ins, tuple(cA), tuple(cB)


def _build(cA, cB, ablate_gather=False, ablate_coll=False, mask_mode="slb",
           no_barrier=True, d1split=False, use_lrelu=False, bufs=3):
    T_ = len(cA)
    cpt = [cA[t] + cB[t] for t in range(T_)]
    cptmax = max(cpt)
    scols = sum(cpt)
    sidx_cols = scols * 8
    # per-tile offsets into the concatenated strips
    off = np.concatenate([[0], np.cumsum(cpt)]).astype(int)

    nc = bacc.Bacc("TRN2", target_bir_lowering=False, debug=False,
                   num_devices=NCORES, num_swdge_queues=4)

    # ---- external inputs (per core) ----
    d_xTf = nc.dram_tensor("xTf", [2, 128, NGPAD], F16, kind="ExternalInput")
    d_xTo = nc.dram_tensor("xTo", [2, 128, NPAD], F16, kind="ExternalInput")
    d_W1 = nc.dram_tensor("W1cat", [128, 2, 48], F16, kind="ExternalInput")
    d_W2 = nc.dram_tensor("W2cat", [128, 2, 49], F16, kind="ExternalInput")
    d_b1 = nc.dram_tensor("b1f", [128, NH], F32, kind="ExternalInput")
    d_b2 = nc.dram_tensor("b2f", [128, C], F32, kind="ExternalInput")
    d_sidx = nc.dram_tensor("sidx", [128, sidx_cols], I16, kind="ExternalInput")
    if mask_mode == "fp8":
        d_masks = nc.dram_tensor("masks", [128, scols * 256], F8,
                                 kind="ExternalInput")
    else:
        d_srcloc = nc.dram_tensor("srcloc2", [128, scols], F32,
                                  kind="ExternalInput")
        d_srcflat = nc.dram_tensor("srcflat2", [1, scols * 128], F16,
                                   kind="ExternalInput")

    d_out = nc.dram_tensor("out", [NPC, C], F32, kind="ExternalOutput")

    # ---- internal DRAM tables ----
    d_t1 = nc.dram_tensor("t1", [N, ROW], F32)              # [h | er | pad]
    d_t2loc = nc.dram_tensor("t2loc", [NPC, ROW], F32)      # [h2 | er2 | pad]
    d_t2 = nc.dram_tensor("t2", [N, ROW], F32, addr_space="Shared")

    groups = [list(range(NCORES))]

    # SWDGE queue must track Tile's DMASW sem-lane round-robin (lane =
    # pool-DMA index % 8, queue = index % 4) or the sem gets updated from a
    # queue it isn't locked to.
    swq = iter(range(1 << 30))

    def gather(out_ap, table_ap, t, half):
        nk = (cA[t] if half == 0 else cB[t])
        o8 = (off[t] + (0 if half == 0 else cA[t])) * 8
        ni = nk * 128
        # single_packet=True wedges the SWDGE ucode above ~1024 idxs
        return nc.gpsimd.dma_gather(
            out_ap=out_ap, in_ap=table_ap,
            idxs_ap=sidx_sb[:, o8:o8 + nk * 8],
            num_idxs=ni, num_idxs_reg=ni, elem_size=ROW,
            single_packet=False, queue_num=next(swq) % 4)

    with tile.TileContext(nc, num_cores=NCORES) as tc:
        with (
            tc.tile_pool(name="const", bufs=1) as cpool,
            tc.tile_pool(name="rt", bufs=1) as rtpool,
            tc.tile_pool(name="work", bufs=bufs) as wp,
            tc.tile_pool(name="small", bufs=bufs + 1) as sp,
            tc.tile_pool(name="psA", bufs=2, space="PSUM") as psA,   # agg
            tc.tile_pool(name="psD", bufs=2, space="PSUM") as psD,   # dense
            tc.tile_pool(name="psT", bufs=2, space="PSUM") as psT,   # transposes
            tc.tile_pool(name="psE", bufs=2, space="PSUM") as psE,   # el expand
        ):
            # ---------- constants ----------
            b1sb = cpool.tile([128, NH], F32)
            nc.sync.dma_start(out=b1sb[:], in_=d_b1.ap())
            b2sb = cpool.tile([128, C], F32)
            nc.sync.dma_start(out=b2sb[:], in_=d_b2.ap())
            ident = cpool.tile([128, 128], F16)
            make_identity(nc, ident[:])
            W1cat = cpool.tile([128, 2, 48], F16)
            nc.sync.dma_start(out=W1cat[:], in_=d_W1.ap())
            W2cat = cpool.tile([128, 2, 49], F16)
            nc.sync.dma_start(out=W2cat[:], in_=d_W2.ap())
            sidx_sb = cpool.tile([128, sidx_cols], I16)
            nc.sync.dma_start(out=sidx_sb[:], in_=d_sidx.ap())
            if mask_mode == "slb":
                iota = cpool.tile([128, 128], F16)
                nc.gpsimd.iota(out=iota[:], pattern=[[1, 128]], base=0,
                               channel_multiplier=0,
                               allow_small_or_imprecise_dtypes=True)
                iotac = cpool.tile([128, 1], F32)
                nc.gpsimd.iota(out=iotac[:], pattern=[[1, 1]], base=0,
                               channel_multiplier=1,
                               allow_small_or_imprecise_dtypes=True)
                srcl_sb = cpool.tile([128, scols], F32)
                nc.sync.dma_start(out=srcl_sb[:], in_=d_srcloc.ap())

            def load_masks(t, ck):
                """Returns (mskT_lhsT(k), msk_lhsT(k)) accessors for tile t."""
                if mask_mode == "fp8":
                    mts = wp.tile([128, cptmax * 256], F8, tag="mts")
                    nc.sync.dma_start(
                        out=mts[:, 0:ck * 256],
                        in_=d_masks.ap()[:, off[t] * 256:off[t] * 256 + ck * 256])
                    return (lambda k: mts[:, k * 128:(k + 1) * 128],
                            lambda k: mts[:, (ck + k) * 128:(ck + k + 1) * 128])
                slb = wp.tile([128, cptmax * 128], F16, tag="slb")
                nc.sync.dma_start(
                    out=slb[:, 0:ck * 128],
                    in_=d_srcflat.ap()[:, off[t] * 128:(off[t] + ck) * 128]
                    .to_broadcast([128, ck * 128]))
                mskT = wp.tile([128, cptmax * 128], F16, tag="mskT")
                nc.vector.tensor_scalar(out=mskT[:, 0:ck * 128],
                                        in0=slb[:, 0:ck * 128],
                                        scalar1=iotac[:],
                                        scalar2=None, op0=ALU.is_equal)
                msk = wp.tile([128, cptmax, 128], F16, tag="msk")
                nc.vector.tensor_tensor(
                    out=msk[:, 0:ck, :],
                    in0=srcl_sb[:, off[t]:off[t] + ck]
                    .rearrange("p k -> p k ()").to_broadcast([128, ck, 128]),
                    in1=iota[:].rearrange("p j -> p () j")
                    .to_broadcast([128, ck, 128]),
                    op=ALU.is_equal)
                return (lambda k: mskT[:, k * 128:(k + 1) * 128],
                        lambda k: msk[:, k, :])

            rT = rtpool.tile([128, 2, NPAD], F16)   # transposed post-elu l1 out
            el2sb = rtpool.tile([128, T_], F32)     # own layer-2 el, per tile

            # ---------- phase D1 (replicated): h/er table for ALL nodes ----
            G4 = 4  # node tiles per group
            ngroups = (TG + G4 - 1) // G4
            for g in range(ngroups):
                tiles = min(G4, TG - g * G4)
                ncols = tiles * 128
                xa = wp.tile([128, 2, G4 * 128], F16, tag="xa")
                nc.sync.dma_start(
                    out=xa[:, :, 0:ncols],
                    in_=d_xTf.ap()[:, :, g * G4 * 128:g * G4 * 128 + ncols]
                    .rearrange("q p n -> p q n"))
                h4 = wp.tile([128, G4, ROW], F32, tag="h4")
                for j in range(tiles):
                    ps = psD.tile([128, 48], F32, tag="dense")
                    for q in range(2):
                        nc.tensor.matmul(out=ps[:],
                                         lhsT=xa[:, q, j * 128:(j + 1) * 128],
                                         rhs=W1cat[:, q, :],
                                         start=q == 0, stop=q == 1)
                    if d1split and j % 2 == 1:
                        nc.vector.tensor_copy(out=h4[:, j, 0:48], in_=ps[:])
                    else:
                        nc.scalar.copy(out=h4[:, j, 0:48], in_=ps[:])
                base = g * G4 * 128
                rows = min(N - base, ncols)
                full_j = rows // 128
                if full_j:
                    nc.sync.dma_start(
                        out=d_t1.ap()[base:base + full_j * 128, 0:48]
                        .rearrange("(j p) f -> p j f", p=128),
                        in_=h4[:, 0:full_j, 0:48])
                rem = rows - full_j * 128
                if rem:
                    nc.sync.dma_start(
                        out=d_t1.ap()[base + full_j * 128:base + rows, 0:48],
                        in_=h4[0:rem, full_j, 0:48])
            if not no_barrier:
                tc.strict_bb_all_engine_barrier()

            # ---------- phase S1 (+ fused D2) ----------
            for t in range(T_):
                rows = 128 if t < T_ - 1 else LAST_ROWS
                ck = cpt[t]

                G1 = wp.tile([128, cptmax, ROW], F32, tag="G1")
                if ablate_gather:
                    nc.sync.dma_start(
                        out=G1[:, 0:ck, :], in_=d_t1.ap()[0:128, :]
                        .rearrange("p f -> p () f").to_broadcast([128, ck, ROW]))
                else:
                    gather(G1[:, 0:cA[t], :], d_t1.ap(), t, 0)
                    gather(G1[:, cA[t]:ck, :], d_t1.ap()[HALF:N, :], t, 1)

                # own el for this tile: el = x @ (W1 @ Wl1)  (cols 40:48)
                xo = sp.tile([128, 2, 128], F16, tag="xo")
                nc.sync.dma_start(
                    out=xo[:],
                    in_=d_xTo.ap()[:, :, t * 128:(t + 1) * 128]
                    .rearrange("q p n -> p q n"))
                pel = psD.tile([128, 8], F32, tag="dense")
                for q in range(2):
                    nc.tensor.matmul(out=pel[:], lhsT=xo[:, q, :],
                                     rhs=W1cat[:, q, 40:48],
                                     start=q == 0, stop=q == 1)
                elt = sp.tile([128, H], F16, tag="elt")
                nc.vector.tensor_copy(out=elt[:], in_=pel[:])

                mskT_at, msk_at = load_masks(t, ck)
                pse = psE.tile([128, cptmax * H], F32, tag="elexp")
                for k in range(ck):
                    nc.tensor.matmul(out=pse[:, k * H:(k + 1) * H],
                                     lhsT=mskT_at(k), rhs=elt[:],
                                     start=k == 0, stop=k == ck - 1,
                                     skip_group_check=True)

                # s = er[dst] + el[src]; leaky; exp
                s = wp.tile([128, cptmax, H], F32, tag="s")
                nc.vector.tensor_tensor(
                    out=s[:, 0:ck, :], in0=G1[:, 0:ck, 32:40],
                    in1=pse[:, 0:ck * H].rearrange("p (k h) -> p k h", h=H),
                    op=ALU.add)
                sL = wp.tile([128, cptmax, H], F32, tag="sL")
                if use_lrelu:
                    nc.scalar.activation(out=sL[:, 0:ck, :], in_=s[:, 0:ck, :],
                                         func=ACT.Lrelu, alpha=0.2)
                else:
                    nc.vector.scalar_tensor_tensor(
                        out=sL[:, 0:ck, :], in0=s[:, 0:ck, :], scalar=0.2,
                        in1=s[:, 0:ck, :], op0=ALU.mult, op1=ALU.max)
                rhs = wp.tile([128, cptmax, 264], F16, tag="rhs")
                nc.scalar.activation(out=rhs[:, 0:ck, 256:264],
                                     in_=sL[:, 0:ck, :], func=ACT.Exp)

                hdb = wp.tile([128, cptmax, F1], F16, tag="hdb")
                nc.scalar.copy(out=hdb[:, 0:ck, :], in_=G1[:, 0:ck, 0:32])

                nc.vector.tensor_tensor(
                    out=rhs[:, 0:ck, 0:256].rearrange(
                        "p k (h f) -> p k h f", h=H),
                    in0=rhs[:, 0:ck, 256:264].rearrange(
                        "p k (h o) -> p k h o", o=1)
                    .to_broadcast([128, ck, H, F1]),
                    in1=hdb[:, 0:ck, :].rearrange(
                        "p k (o f) -> p k o f", o=1)
                    .to_broadcast([128, ck, H, F1]),
                    op=ALU.mult)

                ps1 = psA.tile([128, 264], F32, tag="agg")
                for k in range(ck):
                    nc.tensor.matmul(out=ps1[:], lhsT=msk_at(k),
                                     rhs=rhs[:, k, :],
                                     start=k == 0, stop=k == ck - 1)

                # epilogue: out1 = agg/denom + b1 ; r = elu(out1); rT = r.T
                dn = sp.tile([128, H], F32, tag="dn")
                nc.vector.tensor_scalar(out=dn[:], in0=ps1[:, 256:264],
                                        scalar1=1e-12, scalar2=None, op0=ALU.max)
                rc = sp.tile([128, H], F32, tag="rc")
                nc.vector.reciprocal(out=rc[:], in_=dn[:])
                o1 = wp.tile([128, NH], F32, tag="o1")
                nc.vector.tensor_tensor(
                    out=o1[:].rearrange("p (h f) -> p h f", h=H),
                    in0=ps1[:, 0:256].rearrange("p (h f) -> p h f", h=H),
                    in1=rc[:].rearrange("p (h o) -> p h o", o=1)
                    .to_broadcast([128, H, F1]),
                    op=ALU.mult)
                o1b = wp.tile([128, NH], F32, tag="o1b")
                nc.vector.tensor_tensor(out=o1b[:], in0=o1[:], in1=b1sb[:],
                                        op=ALU.add)
                # elu(x) = max(x,0)-1 + min(exp(x),1)
                ex = wp.tile([128, NH], F32, tag="ex")
                nc.scalar.activation(out=ex[:], in_=o1b[:], func=ACT.Exp)
                p1 = wp.tile([128, NH], F32, tag="p1")
                nc.vector.tensor_scalar(out=p1[:], in0=o1b[:], scalar1=0.0,
                                        scalar2=-1.0, op0=ALU.max, op1=ALU.add)
                r_ = wp.tile([128, NH], F16, tag="r_")
                nc.vector.scalar_tensor_tensor(out=r_[:], in0=ex[:], scalar=1.0,
                                               in1=p1[:], op0=ALU.min,
                                               op1=ALU.add)
                for q in range(2):
                    pt = psT.tile([128, 128], F16, tag="pt")
                    nc.tensor.transpose(out=pt[:], in_=r_[:, q * 128:(q + 1) * 128],
                                        identity=ident[:])
                    if q == 0:
                        nc.scalar.copy(out=rT[:, q, t * 128:(t + 1) * 128],
                                       in_=pt[:])
                    else:
                        nc.vector.tensor_copy(out=rT[:, q, t * 128:(t + 1) * 128],
                                              in_=pt[:])

                # D2: h2/er2/el2 for this tile
                ps2 = psD.tile([128, 49], F32, tag="dense")
                for q in range(2):
                    nc.tensor.matmul(out=ps2[:], lhsT=rT[:, q, t * 128:(t + 1) * 128],
                                     rhs=W2cat[:, q, :], start=q == 0, stop=q == 1)
                h2sb = wp.tile([128, ROW], F32, tag="h2sb")
                nc.scalar.copy(out=h2sb[:, 0:48], in_=ps2[:, 0:48])
                nc.scalar.copy(out=el2sb[:, t:t + 1], in_=ps2[:, 48:49])
                nc.sync.dma_start(out=d_t2loc.ap()[t * 128:t * 128 + rows, 0:48],
                                  in_=h2sb[0:rows, 0:48])

            # ---------- C2: share layer-2 table ----------
            if ablate_coll:
                nc.sync.dma_start(out=d_t2.ap()[0:NPC, :], in_=d_t2loc.ap())
            else:
                nc.gpsimd.collective_compute(
                    "AllGather", ALU.bypass, replica_groups=groups,
                    ins=[d_t2loc.ap()], outs=[d_t2.ap()])
            if not no_barrier:
                tc.strict_bb_all_engine_barrier()

            # ---------- phase S2 ----------
            for t in range(T_):
                rows = 128 if t < T_ - 1 else LAST_ROWS
                ck = cpt[t]

                Gt = wp.tile([128, cptmax, ROW], F32, tag="Gt")
                if ablate_gather:
                    nc.sync.dma_start(
                        out=Gt[:, 0:ck, :], in_=d_t2.ap()[0:128, :]
                        .rearrange("p f -> p () f").to_broadcast([128, ck, ROW]))
                else:
                    gather(Gt[:, 0:cA[t], :], d_t2.ap(), t, 0)
                    gather(Gt[:, cA[t]:ck, :], d_t2.ap()[HALF:N, :], t, 1)

                mskT_at, msk_at = load_masks(t, ck)
                el2t = sp.tile([128, 1], F16, tag="el2t")
                nc.vector.tensor_copy(out=el2t[:], in_=el2sb[:, t:t + 1])
                pse2 = psE.tile([128, cptmax], F32, tag="elexp")
                for k in range(ck):
                    nc.tensor.matmul(out=pse2[:, k:k + 1],
                                     lhsT=mskT_at(k), rhs=el2t[:],
                                     start=k == 0, stop=k == ck - 1,
                                     skip_group_check=True)

                s2 = sp.tile([128, cptmax], F32, tag="s2")
                nc.vector.tensor_tensor(
                    out=s2[:, 0:ck],
                    in0=Gt[:, 0:ck, 47:48].rearrange("p k o -> p (k o)"),
                    in1=pse2[:, 0:ck], op=ALU.add)
                sL2 = sp.tile([128, cptmax], F32, tag="sL2")
                if use_lrelu:
                    nc.scalar.activation(out=sL2[:, 0:ck], in_=s2[:, 0:ck],
                                         func=ACT.Lrelu, alpha=0.2)
                else:
                    nc.vector.scalar_tensor_tensor(
                        out=sL2[:, 0:ck], in0=s2[:, 0:ck], scalar=0.2,
                        in1=s2[:, 0:ck], op0=ALU.mult, op1=ALU.max)
                rhs2 = wp.tile([128, cptmax, 48], F16, tag="rhs2")
                nc.scalar.activation(
                    out=rhs2[:, 0:ck, 47:48].rearrange("p k o -> p (k o)"),
                    in_=sL2[:, 0:ck], func=ACT.Exp)

                nc.vector.tensor_tensor(
                    out=rhs2[:, 0:ck, 0:47], in0=Gt[:, 0:ck, 0:47],
                    in1=rhs2[:, 0:ck, 47:48].to_broadcast([128, ck, C]),
                    op=ALU.mult)
                ps3 = psA.tile([128, 48], F32, tag="agg")
                for k in range(ck):
                    nc.tensor.matmul(out=ps3[:], lhsT=msk_at(k),
                                     rhs=rhs2[:, k, :],
                                     start=k == 0, stop=k == ck - 1)

                # epilogue: out2 = agg2/denom2 + b2, then log_softmax
                dn2 = sp.tile([128, 1], F32, tag="dn2")
                nc.vector.tensor_scalar(out=dn2[:], in0=ps3[:, 47:48],
                                        scalar1=1e-12, scalar2=None, op0=ALU.max)
                rc2 = sp.tile([128, 1], F32, tag="rc2")
                nc.vector.reciprocal(out=rc2[:], in_=dn2[:])
                o2b = wp.tile([128, C], F32, tag="o2b")
                nc.vector.scalar_tensor_tensor(out=o2b[:], in0=ps3[:, 0:47],
                                               scalar=rc2[:, 0:1], in1=b2sb[:],
                                               op0=ALU.mult, op1=ALU.add)
                mx = sp.tile([128, 1], F32, tag="mx")
                nc.vector.tensor_reduce(out=mx[:], in_=o2b[:],
                                        axis=mybir.AxisListType.X, op=ALU.max)
                xm = wp.tile([128, C], F32, tag="xm")
                nc.vector.tensor_scalar(out=xm[:], in0=o2b[:], scalar1=mx[:, 0:1],
                                        scalar2=None, op0=ALU.subtract)
                ex2 = wp.tile([128, C], F32, tag="ex2")
                se = sp.tile([128, 1], F32, tag="se")
                nc.scalar.activation(out=ex2[:], in_=xm[:], func=ACT.Exp,
                                     accum_out=se[:])
                ls = sp.tile([128, 1], F32, tag="ls")
                nc.scalar.activation(out=ls[:], in_=se[:], func=ACT.Ln)
                fin = wp.tile([128, C], F32, tag="fin")
                nc.vector.tensor_scalar(out=fin[:], in0=xm[:], scalar1=ls[:, 0:1],
                                        scalar2=None, op0=ALU.subtract)
                nc.sync.dma_start(out=d_out.ap()[t * 128:t * 128 + rows, :],
                                  in_=fin[0:rows, :])

    nc.compile()
    return nc


def _make_inputs(x, edge_src, edge_dst, W1, Wl1, Wr1, b1, W2, Wl2, Wr2, b2):
    edge_ins, cA, cB = _preprocess(edge_src, edge_dst)
    x = np.asarray(x, dtype=np.float32)
    W1 = np.asarray(W1, dtype=np.float32)
    Wl1 = np.asarray(Wl1, dtype=np.float32)
    Wr1 = np.asarray(Wr1, dtype=np.float32)
    W2 = np.asarray(W2, dtype=np.float32)
    Wl2 = np.asarray(Wl2, dtype=np.float32)
    Wr2 = np.asarray(Wr2, dtype=np.float32)

    W1cat = np.zeros((128, 2, 48), dtype=np.float16)
    W1cat[:, :, 0:32] = W1.reshape(2, 128, F1).transpose(1, 0, 2)
    W1cat[:, :, 32:40] = (W1 @ Wr1).reshape(2, 128, H).transpose(1, 0, 2)
    W1cat[:, :, 40:48] = (W1 @ Wl1).reshape(2, 128, H).transpose(1, 0, 2)
    W2cat = np.zeros((128, 2, 49), dtype=np.float16)
    W2cat[:, :, 0:47] = W2.reshape(2, 128, C).transpose(1, 0, 2)
    W2cat[:, :, 47:48] = (W2 @ Wr2).reshape(2, 128, 1).transpose(1, 0, 2)
    W2cat[:, :, 48:49] = (W2 @ Wl2).reshape(2, 128, 1).transpose(1, 0, 2)

    b1f = np.tile(np.tile(np.asarray(b1, np.float32), H)[None, :], (128, 1))
    b2f = np.tile(np.asarray(b2, np.float32)[None, :], (128, 1))

    xTf = np.zeros((2, 128, NGPAD), dtype=np.float16)
    xTf[:, :, :N] = np.ascontiguousarray(x.T).reshape(2, 128, N)

    common = {
        "xTf": xTf, "W1cat": W1cat, "W2cat": W2cat,
        "b1f": b1f, "b2f": b2f,
    }
    in_maps = []
    for c in range(NCORES):
        xTo = np.zeros((2, 128, NPAD), dtype=np.float16)
        xs = np.ascontiguousarray(x[c * NPC:(c + 1) * NPC].T)   # [256, NPC]
        xTo[:, :, :NPC] = xs.reshape(2, 128, NPC)
        m = dict(common)
        m["xTo"] = xTo
        m.update(edge_ins[c])
        in_maps.append(m)
    return in_maps, (cA, cB)


def _run(inputs, trace=False, **build_kw):
    in_maps, key = _make_inputs(**inputs)
    ck = (key, tuple(sorted(build_kw.items())))
    if ck not in _cache:
        _cache[ck] = _build(*key, **build_kw)
    nc = _cache[ck]
    bkr = run_bass_kernel_spmd(nc, in_maps, list(range(NCORES)), trace=trace)
    out = np.concatenate([bkr.results[c]["out"] for c in range(NCORES)], axis=0)
    return out.astype(np.float32), bkr


def kernel(**inputs):
    out, _ = _run(inputs, trace=False)
    return out
